# revision 1
# baseline (speedup 1.0000x reference)
"""Bahdanau additive attention on 8 Trainium2 NeuronCores.

Data-parallel over batch: core c handles batches [4c, 4c+4).
Per batch b:
  ep[k,t]   = sum_h Ua[k,h] * enc[b,t,h]        (fp32r PE matmuls, k on PSUM partitions)
  z[k,t]    = tanh(ep[k,t] + hp[b,k])           (ScalarE, hp as per-partition bias)
  e[t]      = sum_k va[k] * z[k,t]              (M=1 fp32r PE matmuls)
  attn      = softmax(e) * mask renorm          (DVE/ScalarE on [1,T])
  ctx[h]    = sum_t attn[t] * enc[b,t,h]        (M=1 fp32r PE matmuls, enc natural layout)
hp[b,k] = sum_h Wa[k,h] * h_t[b,h] runs on the DVE (tensor_tensor_reduce of
natural-layout Wa rows against a partition-broadcast h_t row), keeping the PE
free and avoiding any dependency of the first tanh on a big transposed DMA.
Host prep: Ua.T and enc.transpose(0,2,1) give the PE-facing DMAs a contiguous
partition-major layout.
"""

import numpy as np

import concourse.bass as bass
import concourse.tile as tile
from concourse import bacc, mybir

dt = mybir.dt
AF = mybir.ActivationFunctionType

B, T, H = 32, 1024, 1024
NCORES = 8
BL = B // NCORES          # batches per core
P = 128                   # partitions
NT = 512                  # matmul free-dim chunk (one PSUM bank of fp32)
KT = H // P               # k-tiles (output rows of ep)
HT = H // P               # h-tiles (contraction)
TT = T // P               # t-tiles (partition tiles of natural enc)
TC = T // NT              # t chunks per batch

_CACHE = {}


def _build_nc():
    nc = bacc.Bacc("TRN2", target_bir_lowering=False, debug=False)

    # Block layouts (host-prepped) so each DMA is one contiguous transfer
    # delivering exactly what one matmul group consumes:
    #   encT: [BL, TC, H, NT]  tc-major blocks of enc[b].T
    #   encn: [BL, TC, T, NT]  h-chunk-major blocks of enc[b]
    #   uaT:  [KT, H, P]       kt-major column blocks of Ua.T
    #   waT:  [TC, H, NT]      kc-major column blocks of Wa.T
    encT_d = nc.dram_tensor("encT", [BL, TC, P, HT, NT], dt.float32r,
                            kind="ExternalInput").ap()
    encn_d = nc.dram_tensor("encn", [BL, TC, P, TT, NT], dt.float32r,
                            kind="ExternalInput").ap()
    uaT_d = nc.dram_tensor("uaT", [KT, P, HT, P], dt.float32r,
                           kind="ExternalInput").ap()
    waT_d = nc.dram_tensor("waT", [TC, P, HT, NT], dt.float32r,
                           kind="ExternalInput").ap()
    htT_d = nc.dram_tensor("htT", [H, BL], dt.float32r, kind="ExternalInput").ap()
    va_d = nc.dram_tensor("va", [H], dt.float32r, kind="ExternalInput").ap()
    mask_d = nc.dram_tensor("mask", [BL, T], dt.uint8, kind="ExternalInput").ap()

    ctx_d = nc.dram_tensor("ctx", [BL, H], dt.float32, kind="ExternalOutput").ap()
    attn_d = nc.dram_tensor("attn", [BL, T], dt.float32, kind="ExternalOutput").ap()

    with tile.TileContext(nc) as tc:
        from contextlib import ExitStack

        with ExitStack() as st:
            wpool = st.enter_context(tc.tile_pool(name="weights", bufs=1))
            etpool = st.enter_context(tc.tile_pool(name="encT", bufs=4))
            natpool = st.enter_context(tc.tile_pool(name="nat", bufs=4))
            thpool = st.enter_context(tc.tile_pool(name="tanh", bufs=4))
            smpool = st.enter_context(tc.tile_pool(name="small", bufs=1))
            pmain = st.enter_context(tc.tile_pool(name="pmain", bufs=5, space="PSUM"))
            pe_ps = st.enter_context(tc.tile_pool(name="pe", bufs=2, space="PSUM"))
            pctx = st.enter_context(tc.tile_pool(name="pctx", bufs=1, space="PSUM"))

            # ---- tiny constants first (cheap DMAs) ----
            va_sb = wpool.tile([P, KT], dt.float32r, tag="va")
            nc.sync.dma_start(va_sb[:], va_d.rearrange("(kt p) -> p kt", p=P))
            ones_sb = wpool.tile([1, 1], dt.float32, tag="ones")
            nc.vector.memset(ones_sb[:], 1.0)
            ones_r = wpool.tile([1, 1], dt.float32r, tag="ones_r")
            nc.vector.tensor_copy(ones_r[:], ones_sb[:])
            negbig = wpool.tile([1, 1], dt.float32, tag="negbig")
            nc.vector.memset(negbig[:], -1e30)
            hp_sb = wpool.tile([P, KT, BL], dt.float32, tag="hp")
            hpT_sb = wpool.tile([BL, H], dt.float32, tag="hpT")
            htT_sb = wpool.tile([P, HT, BL], dt.float32r, tag="htT")
            nc.sync.dma_start(htT_sb[:], htT_d.rearrange("(ht p) b -> p ht b", p=P))
            ident4 = wpool.tile([BL, BL], dt.float32, tag="ident4")
            from concourse.masks import make_identity
            make_identity(nc, ident4[:])

            # prologue DMAs, in the order the PE needs them
            uaT_sb = [None] * KT

            def load_uaT(kt):
                u = wpool.tile([P, HT, P], dt.float32r, tag=f"uaT{kt}",
                               name=f"uaT{kt}")
                nc.sync.dma_start(u[:], uaT_d[kt])
                uaT_sb[kt] = u

            def load_encT(bi, tcc, split=False):
                t_ = etpool.tile([P, HT, NT], dt.float32r, tag="encT",
                                 name=f"encT{bi}_{tcc}")
                if split:
                    for ht in range(HT):
                        nc.sync.dma_start(t_[:, ht, :], encT_d[bi, tcc, :, ht, :])
                else:
                    nc.sync.dma_start(t_[:], encT_d[bi, tcc])
                return t_

            def load_nat(bi, kc):
                t_ = natpool.tile([P, TT, NT], dt.float32r, tag="nat",
                                  name=f"nat{bi}_{kc}")
                nc.sync.dma_start(t_[:], encn_d[bi, kc])
                return t_

            def load_waT(kc):
                w = natpool.tile([P, HT, NT], dt.float32r, tag="nat",
                                 name=f"waT{kc}")
                for ht in range(HT):
                    nc.sync.dma_start(w[:, ht, :], waT_d[kc, :, ht, :])
                return w
            load_uaT(0)
            encT_b0 = [load_encT(0, 0, split=True)]
            waT_kc = [load_waT(0), load_waT(1)]
            for kt in range(1, KT):
                load_uaT(kt)
            encT_b0.append(load_encT(0, 1, split=True))

            # hp on PE: hpT[b, k] = sum_h htT[h, b] * waT[h, k], then an
            # identity-matmul transpose back to [k partitions, (kt, b)].
            def emit_hp():
                for kc in range(TC):
                    pp = pctx.tile([BL, NT], dt.float32, tag="tailps",
                                   name=f"hp_ps{kc}")
                    for ht in range(HT):
                        nc.tensor.matmul(
                            pp[:], htT_sb[:, ht, :], waT_kc[kc][:, ht, :],
                            start=(ht == 0), stop=(ht == HT - 1))
                    nc.vector.tensor_copy(
                        hpT_sb[:, kc * NT:(kc + 1) * NT], pp[:])
                hpt_ps = pctx.tile([P, KT * BL], dt.float32, tag="tailps")
                for kt in range(KT):
                    nc.tensor.matmul(
                        hpt_ps[:, kt * BL:(kt + 1) * BL],
                        hpT_sb[:, kt * P:(kt + 1) * P], ident4[:],
                        start=True, stop=True)
                nc.vector.tensor_copy(
                    hp_sb[:].rearrange("p kt b -> p (kt b)"), hpt_ps[:])

            def make_tail(bi, e_sb, nat_kc):
                def emit_tail():
                    # softmax with mask folded into the exp product:
                    # attn = (exp(e - max) * m) / sum(exp(e - max) * m)
                    nm = smpool.tile([1, 1], dt.float32, tag="nm")
                    nc.vector.tensor_reduce(nm[:], e_sb[:], axis=mybir.AxisListType.X,
                                            op=mybir.AluOpType.max, negate=True)
                    ex = smpool.tile([1, T], dt.float32, tag="ex")
                    ssum = smpool.tile([1, 1], dt.float32, tag="ssum")
                    nc.scalar.activation(ex[:], e_sb[:], AF.Exp, bias=nm[:],
                                         accum_out=ssum[:])
                    rinv = smpool.tile([1, 1], dt.float32, tag="rinv")
                    nc.vector.reciprocal(rinv[:], ssum[:])
                    attn_sb = smpool.tile([1, T], dt.float32, tag="attn")
                    nc.vector.tensor_scalar_mul(attn_sb[:], ex[:], rinv[:])
                    nc.sync.dma_start(attn_d[bi:bi + 1, :], attn_sb[:])
                    # transpose UNnormalized exp into partitions: [1,T] -> [P,TT]
                    # (context uses ex directly; 1/sum is folded into the final
                    # PSUM->SBUF copy, so ctx does not wait for the reciprocal)
                    atp = pctx.tile([P, TT], dt.float32, tag="tailps")
                    for tt in range(TT):
                        nc.tensor.matmul(
                            atp[:, tt:tt + 1], ex[:, tt * P:(tt + 1) * P],
                            ones_sb[:], start=True, stop=True)
                    attnT = smpool.tile([P, TT], dt.float32r, tag="attnTsb")
                    nc.vector.tensor_copy(attnT[:], atp[:])
                    # context: ctx[h] = (sum_t ex[t] enc[t, h]) / sum(ex)
                    ctx_sb = smpool.tile([1, H], dt.float32, tag="ctx")
                    for kc in range(H // NT):
                        cp = pctx.tile([1, NT], dt.float32, tag="tailps")
                        for tt in range(TT):
                            nc.tensor.matmul(
                                cp[:], attnT[:, tt:tt + 1],
                                nat_kc[kc][:, tt, :],
                                start=(tt == 0), stop=(tt == TT - 1))
                        nc.vector.tensor_scalar_mul(
                            ctx_sb[:, kc * NT:(kc + 1) * NT], cp[:], rinv[:])
                    nc.sync.dma_start(ctx_d[bi:bi + 1, :], ctx_sb[:])
                return emit_tail

            pending_tail = None
            nat_b0 = [None, None]
            encT_b1 = [None, None]

            def b0_prefetch():
                encT_b1[0] = load_encT(1, 0)
                nat_b0[0] = load_nat(0, 0)
                encT_b1[1] = load_encT(1, 1)
                nat_b0[1] = load_nat(0, 1)

            for bi in range(BL):
                if bi == 0:
                    encT_t = encT_b0
                    nat_kc = nat_b0
                elif bi == 1:
                    encT_t = encT_b1
                    nat_kc = [load_nat(bi, kc) for kc in range(TC)]
                else:
                    encT_t = [load_encT(bi, tcc) for tcc in range(TC)]
                    nat_kc = [load_nat(bi, kc) for kc in range(TC)]
                mask_f = smpool.tile([1, T], dt.float32, tag="mask", bufs=1)
                nc.gpsimd.dma_start(mask_f[:], mask_d[bi:bi + 1, :])
                mask_m1 = smpool.tile([1, T], dt.float32r, tag="mask_m1", bufs=1)
                nc.scalar.activation(mask_m1[:], mask_f[:], AF.Identity,
                                     bias=negbig[:], scale=1e30)

                e_ps = [pe_ps.tile([1, NT], dt.float32, tag="e", name=f"e_ps{_}")
                        for _ in range(TC)]
                pending_emms = []
                deferred_finish = []
                gidx = 0

                def finish_group(ps, kt, tcc):
                    th = thpool.tile([P, NT], dt.float32r, tag="th", name="th")
                    nc.scalar.activation(th[:], ps[:], AF.Tanh,
                                         bias=hp_sb[:, kt, bi:bi + 1])
                    def emm():
                        nc.tensor.matmul(
                            e_ps[tcc][:], va_sb[:, kt:kt + 1], th[:],
                            start=(kt == 0), stop=False)
                    pending_emms.append(emm)

                if bi == 0:
                    group_iter = [(kt, tcc) for tcc in range(TC)
                                  for kt in range(KT)]
                else:
                    group_iter = [(kt, tcc) for kt in range(KT)
                                  for tcc in range(TC)]
                for kt, tcc in group_iter:
                    if True:
                        if bi == 0 and gidx == 1:
                            emit_hp()
                            b0_prefetch()
                        if gidx == 2 and pending_tail is not None:
                            pending_tail()
                            pending_tail = None
                        ps = pmain.tile([P, NT], dt.float32, tag="big")
                        for ht in range(HT):
                            nc.tensor.matmul(
                                ps[:], uaT_sb[kt][:, ht, :],
                                encT_t[tcc][:, ht, :],
                                start=(ht == 0), stop=(ht == HT - 1))
                        if bi == 0 and gidx < 1:
                            deferred_finish.append((ps, kt, tcc))
                        else:
                            if deferred_finish:
                                for args in deferred_finish:
                                    finish_group(*args)
                                deferred_finish = []
                            finish_group(ps, kt, tcc)
                        # keep a lag of one group before the e-reduce matmul
                        lag = 0 if bi == BL - 1 and gidx >= 2 * KT - 2 else 1
                        while len(pending_emms) > lag:
                            pending_emms.pop(0)()
                        gidx += 1
                for f in pending_emms:
                    f()
                for tcc in range(TC):
                    nc.tensor.matmul(
                        e_ps[tcc][:], ones_r[:],
                        mask_m1[:, tcc * NT:(tcc + 1) * NT],
                        start=False, stop=True)
                # e chunks -> SBUF (frees e psum slots early)
                e_sb = smpool.tile([1, T], dt.float32, tag="e_sb", bufs=2)
                for tcc in range(TC):
                    nc.vector.tensor_copy(e_sb[:, tcc * NT:(tcc + 1) * NT],
                                          e_ps[tcc][:])
                pending_tail = make_tail(bi, e_sb, nat_kc)
            pending_tail()

    nc.compile()
    return nc


def _get_runner():
    if "runner" in _CACHE:
        return _CACHE["runner"]

    import jax
    from jax.sharding import Mesh, PartitionSpec
    from jax.experimental.shard_map import shard_map
    from concourse import bass2jax
    from concourse import mybir as _mb

    nc = _build_nc()
    bass2jax.install_neuronx_cc_hook()

    partition_name = (nc.partition_id_tensor.name
                      if nc.partition_id_tensor else None)
    in_names, out_names, out_avals, zero_outs = [], [], [], []
    for alloc in nc.m.functions[0].allocations:
        if not isinstance(alloc, _mb.MemoryLocationSet):
            continue
        name = alloc.memorylocations[0].name
        if alloc.kind == "ExternalInput":
            if name != partition_name:
                in_names.append(name)
        elif alloc.kind == "ExternalOutput":
            out_names.append(name)
            shape = tuple(alloc.tensor_shape)
            npdt = _mb.dt.np(alloc.dtype)
            out_avals.append(jax.core.ShapedArray(shape, npdt))
            zero_outs.append(np.zeros(shape, npdt))
    n_params = len(in_names)
    n_outs = len(out_names)
    all_in_names = in_names + out_names
    if partition_name is not None:
        all_in_names = all_in_names + [partition_name]
    donate = tuple(range(n_params, n_params + n_outs))

    def _body(*args):
        operands = list(args)
        if partition_name is not None:
            operands.append(bass2jax.partition_id_tensor())
        outs = bass2jax._bass_exec_p.bind(
            *operands,
            out_avals=tuple(out_avals),
            in_names=tuple(all_in_names),
            out_names=tuple(out_names),
            lowering_input_output_aliases=(),
            sim_require_finite=True,
            sim_require_nnan=True,
            nc=nc,
        )
        return tuple(outs)

    devices = jax.devices()[:NCORES]
    mesh = Mesh(np.asarray(devices), ("core",))
    in_specs = (PartitionSpec("core"),) * (n_params + n_outs)
    out_specs = (PartitionSpec("core"),) * n_outs
    sharded = jax.jit(
        shard_map(_body, mesh=mesh, in_specs=in_specs, out_specs=out_specs,
                  check_rep=False),
        donate_argnums=donate, keep_unused=True)

    def run(in_maps):
        concat_in = [
            np.concatenate([np.asarray(m[name]) for m in in_maps], axis=0)
            for name in in_names
        ]
        concat_zeros = [
            np.zeros((NCORES * z.shape[0], *z.shape[1:]), z.dtype)
            for z in zero_outs
        ]
        out_arrs = sharded(*concat_in, *concat_zeros)
        return [
            {name: np.asarray(out_arrs[i]).reshape(NCORES, *out_avals[i].shape)[c]
             for i, name in enumerate(out_names)}
            for c in range(NCORES)
        ]

    _CACHE["runner"] = run
    return run


def _make_in_maps(inputs):
    h_t = np.asarray(inputs["h_t"], dtype=np.float32)
    enc_out = np.asarray(inputs["enc_out"], dtype=np.float32)
    src_mask = np.asarray(inputs["src_mask"])
    Wa = np.asarray(inputs["Wa"], dtype=np.float32)
    Ua = np.asarray(inputs["Ua"], dtype=np.float32)
    va = np.asarray(inputs["va"], dtype=np.float32)

    uaT = np.ascontiguousarray(
        Ua.T.reshape(HT, P, KT, P).transpose(2, 1, 0, 3))    # [KT, P, HT, P]
    waT = np.ascontiguousarray(
        Wa.T.reshape(HT, P, TC, NT).transpose(2, 1, 0, 3))   # [TC, P, HT, NT]
    htT = np.ascontiguousarray(h_t.T)                        # [H, B]
    encT = np.ascontiguousarray(
        enc_out.transpose(0, 2, 1).reshape(B, HT, P, TC, NT)
        .transpose(0, 3, 2, 1, 4))                           # [B, TC, P, HT, NT]
    encn = np.ascontiguousarray(
        enc_out.reshape(B, TT, P, TC, NT)
        .transpose(0, 3, 2, 1, 4))                           # [B, TC, P, TT, NT]
    mask_u8 = np.ascontiguousarray(src_mask.astype(np.uint8))

    in_maps = []
    for c in range(NCORES):
        sl = slice(c * BL, (c + 1) * BL)
        in_maps.append({
            "encT": encT[sl],
            "encn": encn[sl],
            "uaT": uaT,
            "waT": waT,
            "htT": np.ascontiguousarray(htT[:, sl]),
            "va": va,
            "mask": mask_u8[sl],
        })
    return in_maps


def kernel(h_t, enc_out, src_mask, Wa, Ua, va):
    in_maps = _make_in_maps({
        "h_t": h_t, "enc_out": enc_out, "src_mask": src_mask,
        "Wa": Wa, "Ua": Ua, "va": va,
    })
    run = _get_runner()
    results = run(in_maps)
    context = np.concatenate([r["ctx"] for r in results], axis=0)
    attn = np.concatenate([r["attn"] for r in results], axis=0)
    return context, attn



# revision 8
# speedup vs baseline: 1.2663x; 1.2663x over previous
"""Bahdanau additive attention on 8 Trainium2 NeuronCores.

Data-parallel over batch: core c handles batches [4c, 4c+4).
Per batch b:
  ep[k,t]   = sum_h Ua[k,h] * enc[b,t,h]        (bf16 PE matmuls, k on PSUM partitions)
  z[k,t]    = tanh(ep[k,t] + hp[b,k])           (ScalarE, hp as per-partition bias)
  e[t]      = sum_k va[k] * z[k,t]              (M=1 bf16 PE matmuls, 4x col-packed)
  attn      = softmax(e) * mask renorm          (DVE/ScalarE on [1,T]; no max-sub, e is bounded)
  ctx[h]    = sum_t attn[t] * enc[b,t,h]        (M=1 bf16 PE matmuls, 4x col-packed)
hp[b,k] = sum_h Wa[k,h] * h_t[b,h] runs on the PE as two M=4 matmul chains
(one per 512-wide k chunk) + tiny identity-transposes, interleaved with the
first main-matmul groups so no tanh is ever deferred.
Host prep: everything PE-facing is bf16 (2x matmul rate vs fp32r, half DMA).
"""

import numpy as np

import concourse.bass as bass
import concourse.tile as tile
from concourse import bacc, mybir

dt = mybir.dt
AF = mybir.ActivationFunctionType

B, T, H = 32, 1024, 1024
NCORES = 8
BL = B // NCORES          # batches per core
P = 128                   # partitions
NT = 512                  # matmul free-dim chunk (one PSUM bank of fp32)
KT = H // P               # k-tiles (output rows of ep)
HT = H // P               # h-tiles (contraction)
TT = T // P               # t-tiles (partition tiles of natural enc)
TC = T // NT              # t chunks per batch
EC = 4                    # e/ctx col-packed chunks
EW = T // EC              # 256

_CACHE = {}


def _build_nc():
    nc = bacc.Bacc("TRN2", target_bir_lowering=False, debug=False)

    # Block layouts (host-prepped, all bf16) so each DMA is one contiguous
    # transfer delivering exactly what one matmul group consumes:
    #   encT: [BL, TC, H, NT]  tc-major blocks of enc[b].T
    #   encn: [BL, TC, T, NT]  h-chunk-major blocks of enc[b]
    #   uaT:  [KT, P, HT, P]   kt-major column blocks of Ua.T
    #   waT:  [TC, P, HT, NT]  kc-major column blocks of Wa.T
    encT_d = nc.dram_tensor("encT", [BL, TC, P, HT, NT], dt.bfloat16,
                            kind="ExternalInput").ap()
    encn_d = nc.dram_tensor("encn", [BL, TC, P, TT, NT], dt.bfloat16,
                            kind="ExternalInput").ap()
    uaT_d = nc.dram_tensor("uaT", [KT, P, HT, P], dt.bfloat16,
                           kind="ExternalInput").ap()
    waT_d = nc.dram_tensor("waT", [TC, P, HT, NT], dt.bfloat16,
                           kind="ExternalInput").ap()
    htT_d = nc.dram_tensor("htT", [H, BL], dt.bfloat16, kind="ExternalInput").ap()
    va_d = nc.dram_tensor("va", [H], dt.bfloat16, kind="ExternalInput").ap()
    mask_d = nc.dram_tensor("mask", [BL, T], dt.uint8, kind="ExternalInput").ap()

    ctx_d = nc.dram_tensor("ctx", [BL, H], dt.float32, kind="ExternalOutput").ap()
    attn_d = nc.dram_tensor("attn", [BL, T], dt.float32, kind="ExternalOutput").ap()

    with tile.TileContext(nc) as tc:
        from contextlib import ExitStack

        with ExitStack() as st:
            wpool = st.enter_context(tc.tile_pool(name="weights", bufs=1))
            etpool = st.enter_context(tc.tile_pool(name="encT", bufs=4))
            natpool = st.enter_context(tc.tile_pool(name="nat", bufs=4))
            thpool = st.enter_context(tc.tile_pool(name="tanh", bufs=4))
            smpool = st.enter_context(tc.tile_pool(name="small", bufs=1))
            pmain = st.enter_context(tc.tile_pool(name="pmain", bufs=4, space="PSUM"))
            pe_ps = st.enter_context(tc.tile_pool(name="pe", bufs=2, space="PSUM"))
            ptail = st.enter_context(tc.tile_pool(name="ptail", bufs=2, space="PSUM"))

            # ---- tiny constants first (cheap DMAs) ----
            va_sb = wpool.tile([P, KT], dt.bfloat16, tag="va")
            nc.sync.dma_start(va_sb[:], va_d.rearrange("(kt p) -> p kt", p=P))
            htT_sb = wpool.tile([P, HT, BL], dt.bfloat16, tag="htT")
            nc.sync.dma_start(htT_sb[:], htT_d.rearrange("(ht p) b -> p ht b", p=P))
            ones_b = wpool.tile([1, 1], dt.bfloat16, tag="ones_b")
            nc.vector.memset(ones_b[:], 1.0)
            negbig4 = wpool.tile([BL, 1], dt.float32, tag="negbig")
            nc.vector.memset(negbig4[:], -1e30)
            hp_sb = wpool.tile([P, KT, BL], dt.float32, tag="hp")
            ident4 = wpool.tile([BL, BL], dt.bfloat16, tag="ident4")
            from concourse.masks import make_identity
            make_identity(nc, ident4[:])

            # ---- DMA helpers; emission order == queue order ----
            uaT_sb = [None] * KT
            waT_t = [None] * TC

            def load_uaT(kt):
                u = wpool.tile([P, HT, P], dt.bfloat16, tag=f"uaT{kt}",
                               name=f"uaT{kt}")
                nc.sync.dma_start(u[:], uaT_d[kt])
                uaT_sb[kt] = u

            def load_waT(kc, split=False):
                w = natpool.tile([P, HT, NT], dt.bfloat16, tag="nat",
                                 name=f"waT{kc}")
                if split:
                    for ht in range(HT):
                        nc.sync.dma_start(w[:, ht, :], waT_d[kc, :, ht, :])
                else:
                    nc.sync.dma_start(w[:], waT_d[kc])
                waT_t[kc] = w

            def load_encT(bi, tcc, split=False):
                t_ = etpool.tile([P, HT, NT], dt.bfloat16, tag="encT",
                                 name=f"encT{bi}_{tcc}")
                if split:
                    for ht in range(HT):
                        nc.sync.dma_start(t_[:, ht, :], encT_d[bi, tcc, :, ht, :])
                else:
                    nc.sync.dma_start(t_[:], encT_d[bi, tcc])
                return t_

            def load_nat(bi, kc):
                t_ = natpool.tile([P, TT, NT], dt.bfloat16, tag="nat",
                                  name=f"nat{bi}_{kc}")
                nc.sync.dma_start(t_[:], encn_d[bi, kc])
                return t_

            # prologue DMAs, in the order the PE needs them
            load_waT(0, split=True)
            load_uaT(0)
            encT_b0 = [load_encT(0, 0, split=True)]
            load_waT(1)
            for kt in range(1, KT):
                load_uaT(kt)
            encT_b0.append(load_encT(0, 1, split=True))

            # mask for all batches in one cast-DMA + one activation, kept on
            # partition 0 (matmul operands must be 32-aligned in partitions):
            # mask_m1 = (mask - 1) * 1e30  (0 where kept, -1e30 where masked)
            mask_f = smpool.tile([1, BL * T], dt.float32, tag="mask")
            nc.gpsimd.dma_start(mask_f[:], mask_d.rearrange("b t -> (b t)"))
            mask_m1 = smpool.tile([1, BL * T], dt.bfloat16, tag="mask_m1")
            nc.scalar.activation(mask_m1[:], mask_f[:], AF.Identity,
                                 bias=negbig4[0:1, :], scale=1e30)

            # hp for k-chunk kc (4 kt tiles): hpT[b, k] = sum_h htT[h, b] *
            # waT[h, k] (M=4 matmuls, full-rate), then tiny identity-matmul
            # transposes back to [k partitions, (kt, b)].
            def emit_hp(kc):
                hpT_ps = ptail.tile([BL, NT], dt.float32, tag="tailps",
                                    name=f"hpT_ps{kc}")
                for ht in range(HT):
                    nc.tensor.matmul(hpT_ps[:], htT_sb[:, ht, :],
                                     waT_t[kc][:, ht, :],
                                     start=(ht == 0), stop=(ht == HT - 1))
                hpT_sb = smpool.tile([BL, NT], dt.bfloat16, tag="hpT", bufs=2,
                                     name=f"hpT_sb{kc}")
                nc.vector.tensor_copy(hpT_sb[:], hpT_ps[:])
                hp_ps = ptail.tile([P, 4 * BL], dt.float32, tag="tailps",
                                   name=f"hp_ps{kc}")
                for k4 in range(4):
                    nc.tensor.matmul(hp_ps[:, k4 * BL:(k4 + 1) * BL],
                                     hpT_sb[:, k4 * P:(k4 + 1) * P],
                                     ident4[:], start=True, stop=True)
                nc.vector.tensor_copy(
                    hp_sb[:, kc * 4:(kc + 1) * 4, :].rearrange(
                        "p a b -> p (a b)"), hp_ps[:])

            # ---- deferred post-op FIFO: one item emitted per main group ----
            post_q = []

            def pop_post():
                if post_q:
                    post_q.pop(0)()

            def make_epack(e_tile, bi, kt, chunks, th_of):
                def emit():
                    for c in chunks:
                        nc.tensor.matmul(
                            e_tile[32 * c:32 * c + 1, :EW],
                            va_sb[:, kt:kt + 1],
                            th_of[c // TC][:, (c % TC) * EW:(c % TC + 1) * EW],
                            start=(kt == 0), stop=False,
                            tile_position=(0, 32 * c))
                return emit

            def make_mask_and_copy(e_tile, e_sb, bi):
                def emit():
                    for c in range(EC):
                        nc.tensor.matmul(
                            e_tile[32 * c:32 * c + 1, :EW],
                            ones_b[:],
                            mask_m1[0:1, bi * T + c * EW:bi * T + (c + 1) * EW],
                            start=False, stop=True,
                            tile_position=(0, 32 * c))
                    for c in range(EC):
                        nc.vector.tensor_copy(e_sb[:, c * EW:(c + 1) * EW],
                                              e_tile[32 * c:32 * c + 1, :EW])
                return emit

            def make_tail(bi, e_sb, nat_kc):
                def emit():
                    # softmax without max-subtraction (|e| <~ 6 is safe in
                    # fp32); masked lanes hold -1e30 -> exp gives 0.
                    ex = smpool.tile([1, T], dt.bfloat16, tag="ex", bufs=2)
                    ssum = smpool.tile([1, 1], dt.float32, tag="ssum", bufs=2)
                    nc.scalar.activation(ex[:], e_sb[:], AF.Exp,
                                         accum_out=ssum[:])
                    rinv = smpool.tile([1, 1], dt.float32, tag="rinv", bufs=2)
                    nc.vector.reciprocal(rinv[:], ssum[:])
                    attn_sb = smpool.tile([1, T], dt.float32, tag="attn",
                                          bufs=2)
                    nc.vector.tensor_scalar_mul(attn_sb[:], ex[:], rinv[:])
                    nc.sync.dma_start(attn_d[bi:bi + 1, :], attn_sb[:])
                    # transpose UNnormalized exp into partitions: [1,T]->[P,TT]
                    atp = ptail.tile([P, TT], dt.float32, tag="tailps",
                                     name=f"atp{bi}")
                    for tt in range(TT):
                        nc.tensor.matmul(
                            atp[:, tt:tt + 1], ex[:, tt * P:(tt + 1) * P],
                            ones_b[:], start=True, stop=True)
                    attnT = smpool.tile([P, TT], dt.bfloat16, tag="attnT",
                                        bufs=2)
                    nc.vector.tensor_copy(attnT[:], atp[:])
                    # context: ctx[h] = (sum_t ex[t] enc[t, h]) / sum(ex),
                    # 4 col-packed M=1 chains over the natural-layout enc
                    cp = ptail.tile([P, NT], dt.float32, tag="tailps",
                                    name=f"cp{bi}")
                    for tt in range(TT):
                        for c in range(EC):
                            nc.tensor.matmul(
                                cp[32 * c:32 * c + 1, :EW],
                                attnT[:, tt:tt + 1],
                                nat_kc[c // TC][:, tt,
                                                (c % TC) * EW:(c % TC + 1) * EW],
                                start=(tt == 0), stop=(tt == TT - 1),
                                tile_position=(0, 32 * c))
                    ctx_sb = smpool.tile([1, H], dt.float32, tag="ctx", bufs=2)
                    for c in range(EC):
                        nc.vector.tensor_scalar_mul(
                            ctx_sb[:, c * EW:(c + 1) * EW],
                            cp[32 * c:32 * c + 1, :EW], rinv[:])
                    nc.sync.dma_start(ctx_d[bi:bi + 1, :], ctx_sb[:])
                return emit

            # ---- main loop ----
            emit_hp(0)
            for bi in range(BL):
                if bi == 0:
                    encT_t = encT_b0
                    group_iter = [(kt, tcc) for tcc in range(TC)
                                  for kt in range(KT)]
                else:
                    # prefetches for this batch's tail and the next batch
                    group_iter = [(kt, tcc) for kt in range(KT)
                                  for tcc in range(TC)]
                e_tile = pe_ps.tile([P, NT], dt.float32, tag="e",
                                    name=f"e_ps{bi}")
                e_sb = smpool.tile([1, T], dt.float32, tag="e_sb", bufs=2,
                                   name=f"e_sb{bi}")
                th_of = {}
                for gi, (kt, tcc) in enumerate(group_iter):
                    if bi == 0 and gi == 1:
                        # b0 prefetches (b0's nat + b1's encT)
                        encT_b1 = [load_encT(1, 0), load_encT(1, 1)]
                        nat_kc = [load_nat(0, 0), load_nat(0, 1)]
                    if bi > 0 and gi == 0:
                        if bi == 1:
                            encT_t = encT_b1
                        else:
                            encT_t = encT_next
                        if bi < BL - 1:
                            encT_next = [load_encT(bi + 1, 0),
                                         load_encT(bi + 1, 1)]
                        nat_kc = [load_nat(bi, 0), load_nat(bi, 1)]
                    ps = pmain.tile([P, NT], dt.float32, tag="big")
                    for ht in range(HT):
                        nc.tensor.matmul(
                            ps[:], uaT_sb[kt][:, ht, :],
                            encT_t[tcc][:, ht, :],
                            start=(ht == 0), stop=(ht == HT - 1))
                    th = thpool.tile([P, NT], dt.bfloat16, tag="th", name="th")
                    nc.scalar.activation(th[:], ps[:], AF.Tanh,
                                         bias=hp_sb[:, kt, bi:bi + 1])
                    th_of[(kt, tcc)] = th
                    pop_post()
                    if bi == 0:
                        post_q.append(make_epack(
                            e_tile, bi, kt, [2 * tcc, 2 * tcc + 1],
                            {tcc: th}))
                        if gi == 0:
                            post_q.append(lambda: emit_hp(1))
                    elif tcc == 1:
                        post_q.append(make_epack(
                            e_tile, bi, kt, [0, 1, 2, 3],
                            {0: th_of[(kt, 0)], 1: th}))
                post_q.append(make_mask_and_copy(e_tile, e_sb, bi))
                post_q.append(make_tail(bi, e_sb, nat_kc))
            while post_q:
                post_q.pop(0)()

    nc.compile()
    return nc


def _get_runner():
    if "runner" in _CACHE:
        return _CACHE["runner"]

    import jax
    from jax.sharding import Mesh, PartitionSpec
    from jax.experimental.shard_map import shard_map
    from concourse import bass2jax
    from concourse import mybir as _mb

    nc = _build_nc()
    bass2jax.install_neuronx_cc_hook()

    partition_name = (nc.partition_id_tensor.name
                      if nc.partition_id_tensor else None)
    in_names, out_names, out_avals, zero_outs = [], [], [], []
    for alloc in nc.m.functions[0].allocations:
        if not isinstance(alloc, _mb.MemoryLocationSet):
            continue
        name = alloc.memorylocations[0].name
        if alloc.kind == "ExternalInput":
            if name != partition_name:
                in_names.append(name)
        elif alloc.kind == "ExternalOutput":
            out_names.append(name)
            shape = tuple(alloc.tensor_shape)
            npdt = _mb.dt.np(alloc.dtype)
            out_avals.append(jax.core.ShapedArray(shape, npdt))
            zero_outs.append(np.zeros(shape, npdt))
    n_params = len(in_names)
    n_outs = len(out_names)
    all_in_names = in_names + out_names
    if partition_name is not None:
        all_in_names = all_in_names + [partition_name]
    donate = tuple(range(n_params, n_params + n_outs))

    def _body(*args):
        operands = list(args)
        if partition_name is not None:
            operands.append(bass2jax.partition_id_tensor())
        outs = bass2jax._bass_exec_p.bind(
            *operands,
            out_avals=tuple(out_avals),
            in_names=tuple(all_in_names),
            out_names=tuple(out_names),
            lowering_input_output_aliases=(),
            sim_require_finite=True,
            sim_require_nnan=True,
            nc=nc,
        )
        return tuple(outs)

    devices = jax.devices()[:NCORES]
    mesh = Mesh(np.asarray(devices), ("core",))
    in_specs = (PartitionSpec("core"),) * (n_params + n_outs)
    out_specs = (PartitionSpec("core"),) * n_outs
    sharded = jax.jit(
        shard_map(_body, mesh=mesh, in_specs=in_specs, out_specs=out_specs,
                  check_rep=False),
        donate_argnums=donate, keep_unused=True)

    def run(in_maps):
        concat_in = [
            np.concatenate([np.asarray(m[name]) for m in in_maps], axis=0)
            for name in in_names
        ]
        concat_zeros = [
            np.zeros((NCORES * z.shape[0], *z.shape[1:]), z.dtype)
            for z in zero_outs
        ]
        out_arrs = sharded(*concat_in, *concat_zeros)
        return [
            {name: np.asarray(out_arrs[i]).reshape(NCORES, *out_avals[i].shape)[c]
             for i, name in enumerate(out_names)}
            for c in range(NCORES)
        ]

    _CACHE["runner"] = run
    return run


def _make_in_maps(inputs):
    import ml_dtypes
    bf16 = ml_dtypes.bfloat16

    h_t = np.asarray(inputs["h_t"], dtype=np.float32)
    enc_out = np.asarray(inputs["enc_out"], dtype=np.float32)
    src_mask = np.asarray(inputs["src_mask"])
    Wa = np.asarray(inputs["Wa"], dtype=np.float32)
    Ua = np.asarray(inputs["Ua"], dtype=np.float32)
    va = np.asarray(inputs["va"], dtype=np.float32)

    uaT = np.ascontiguousarray(
        Ua.T.reshape(HT, P, KT, P).transpose(2, 1, 0, 3)).astype(bf16)
    waT = np.ascontiguousarray(
        Wa.T.reshape(HT, P, TC, NT).transpose(2, 1, 0, 3)).astype(bf16)
    htT = np.ascontiguousarray(h_t.T).astype(bf16)              # [H, B]
    encT = np.ascontiguousarray(
        enc_out.transpose(0, 2, 1).reshape(B, HT, P, TC, NT)
        .transpose(0, 3, 2, 1, 4)).astype(bf16)                 # [B, TC, P, HT, NT]
    encn = np.ascontiguousarray(
        enc_out.reshape(B, TT, P, TC, NT)
        .transpose(0, 3, 2, 1, 4)).astype(bf16)                 # [B, TC, P, TT, NT]
    va_b = va.astype(bf16)
    mask_u8 = np.ascontiguousarray(src_mask.astype(np.uint8))

    in_maps = []
    for c in range(NCORES):
        sl = slice(c * BL, (c + 1) * BL)
        in_maps.append({
            "encT": encT[sl],
            "encn": encn[sl],
            "uaT": uaT,
            "waT": waT,
            "htT": np.ascontiguousarray(htT[:, sl]),
            "va": va_b,
            "mask": mask_u8[sl],
        })
    return in_maps


def kernel(h_t, enc_out, src_mask, Wa, Ua, va):
    in_maps = _make_in_maps({
        "h_t": h_t, "enc_out": enc_out, "src_mask": src_mask,
        "Wa": Wa, "Ua": Ua, "va": va,
    })
    run = _get_runner()
    results = run(in_maps)
    context = np.concatenate([r["ctx"] for r in results], axis=0)
    attn = np.concatenate([r["attn"] for r in results], axis=0)
    return context, attn


# revision 10
# speedup vs baseline: 1.5201x; 1.2004x over previous
"""Bahdanau additive attention on 8 Trainium2 NeuronCores.

Data-parallel over batch: core c handles batches [4c, 4c+4).
Per batch b:
  ep[k,t]   = sum_h Ua[k,h] * enc[b,t,h]        (bf16 PE matmuls, k on PSUM partitions)
  z[k,t]    = tanh(ep[k,t] + hp[b,k])           (ScalarE, hp as per-partition bias)
  e[t]      = sum_k va[k] * z[k,t]              (M=1 bf16 PE matmuls, 4x col-packed)
  attn      = softmax(e) * mask renorm          (strip-exp out of PSUM; no max-sub)
  ctx[h]    = sum_t attn[t] * enc[b,t,h]        (M=1 bf16 PE matmuls, 4x col-packed,
                                                 1/sum folded into the exp-transpose)
hp[b,k] = sum_h Wa[k,h] * h_t[b,h] runs as per-kt N=4 matmul chains whose tiny
weight chunks trickle in with the staged weight DMAs, filling the DMA-starved
prologue. All one-time weights (Ua blocks, Wa blocks, va, h_t^T) are packed into
a single [128, WX] DRAM tensor loaded with a handful of staged DMAs (descriptor
issue on the Sync engine costs ~0.7us per dma_start).
"""

import numpy as np

import concourse.bass as bass
import concourse.tile as tile
from concourse import bacc, mybir

dt = mybir.dt
AF = mybir.ActivationFunctionType

B, T, H = 32, 1024, 1024
NCORES = 8
BL = B // NCORES          # batches per core
P = 128                   # partitions
NT = 512                  # matmul free-dim chunk (one PSUM bank of fp32)
KT = H // P               # k-tiles (output rows of ep)
HT = H // P               # h-tiles (contraction)
TT = T // P               # t-tiles (partition tiles of natural enc)
TC = T // NT              # t chunks per batch
EC = 4                    # e/ctx col-packed chunks
EW = T // EC              # 256

# packed-weight column offsets (bf16 elements per partition)
OFF_UA0 = 0
OFF_VA = 1024
OFF_HTT = 1032
OFF_WA0 = 1064
OFF_REST = 2088           # kt>=1: [uaT_kt (1024) | waT_kt (1024)] blocks
WX = OFF_REST + (KT - 1) * 2048


def _off_ua(kt):
    return OFF_UA0 if kt == 0 else OFF_REST + (kt - 1) * 2048


def _off_wa(kt):
    return OFF_WA0 if kt == 0 else OFF_REST + (kt - 1) * 2048 + 1024


_CACHE = {}


def _build_nc():
    nc = bacc.Bacc("TRN2", target_bir_lowering=False, debug=False)

    encT_d = nc.dram_tensor("encT", [BL, TC, P, HT, NT], dt.bfloat16,
                            kind="ExternalInput").ap()
    encn_d = nc.dram_tensor("encn", [BL, TC, P, TT, NT], dt.bfloat16,
                            kind="ExternalInput").ap()
    wall_d = nc.dram_tensor("wall", [P, WX], dt.bfloat16,
                            kind="ExternalInput").ap()
    mask_d = nc.dram_tensor("mask", [BL, T], dt.uint8, kind="ExternalInput").ap()

    ctx_d = nc.dram_tensor("ctx", [BL, H], dt.float32, kind="ExternalOutput").ap()
    attn_d = nc.dram_tensor("attn", [BL, T], dt.float32, kind="ExternalOutput").ap()

    with tile.TileContext(nc) as tc:
        from contextlib import ExitStack

        with ExitStack() as st:
            wpool = st.enter_context(tc.tile_pool(name="weights", bufs=1))
            etpool = st.enter_context(tc.tile_pool(name="encT", bufs=4))
            natpool = st.enter_context(tc.tile_pool(name="nat", bufs=4))
            thpool = st.enter_context(tc.tile_pool(name="tanh", bufs=4))
            smpool = st.enter_context(tc.tile_pool(name="small", bufs=1))
            pmain = st.enter_context(tc.tile_pool(name="pmain", bufs=4, space="PSUM"))
            pe_ps = st.enter_context(tc.tile_pool(name="pe", bufs=2, space="PSUM"))
            ptail = st.enter_context(tc.tile_pool(name="ptail", bufs=2, space="PSUM"))

            wall_sb = wpool.tile([P, WX], dt.bfloat16, tag="wall")

            def uaT_ap(kt, ht):
                o = _off_ua(kt) + ht * P
                return wall_sb[:, o:o + P]

            def waT_ap(kt, ht):
                o = _off_wa(kt) + ht * P
                return wall_sb[:, o:o + P]

            def htT_ap(ht):
                o = OFF_HTT + ht * BL
                return wall_sb[:, o:o + BL]

            def va_ap(kt):
                return wall_sb[:, OFF_VA + kt:OFF_VA + kt + 1]

            def load_wall(c0, c1):
                nc.sync.dma_start(wall_sb[:, c0:c1], wall_d[:, c0:c1])

            def load_encT(bi, tcc, split=False):
                t_ = etpool.tile([P, HT, NT], dt.bfloat16, tag="encT",
                                 name=f"encT{bi}_{tcc}")
                if split:
                    nc.sync.dma_start(t_[:, 0:4, :], encT_d[bi, tcc, :, 0:4, :])
                    nc.sync.dma_start(t_[:, 4:8, :], encT_d[bi, tcc, :, 4:8, :])
                else:
                    nc.sync.dma_start(t_[:], encT_d[bi, tcc])
                return t_

            def load_nat(bi, kc):
                t_ = natpool.tile([P, TT, NT], dt.bfloat16, tag="nat",
                                  name=f"nat{bi}_{kc}")
                nc.scalar.dma_start(t_[:], encn_d[bi, kc])
                return t_

            # prologue DMAs, need-ordered (sync queue)
            load_wall(OFF_UA0, OFF_VA)            # uaT0
            encT_b0 = [load_encT(0, 0, split=True)]
            load_wall(OFF_VA, OFF_REST)           # va + htT + waT0
            encT_b0.append(load_encT(0, 1))
            load_wall(OFF_REST, _off_ua(3))       # kt1, kt2
            load_wall(_off_ua(3), _off_ua(5))     # kt3, kt4
            load_wall(_off_ua(5), WX)             # kt5..7

            ones_b = wpool.tile([1, 1], dt.bfloat16, tag="ones_b")
            nc.vector.memset(ones_b[:], 1.0)
            negbig = wpool.tile([1, 1], dt.float32, tag="negbig")
            nc.vector.memset(negbig[:], -1e30)
            hp_sb = wpool.tile([P, KT, BL], dt.float32, tag="hp")

            # mask for all batches in one cast-DMA + one activation, on
            # partition 0: mask_m1 = (mask - 1) * 1e30
            mask_f = smpool.tile([1, BL * T], dt.float32, tag="mask")
            nc.gpsimd.dma_start(mask_f[:], mask_d.rearrange("b t -> (b t)"))
            mask_m1 = smpool.tile([1, BL * T], dt.bfloat16, tag="mask_m1")
            nc.scalar.activation(mask_m1[:], mask_f[:], AF.Identity,
                                 bias=negbig[:], scale=1e30)

            def emit_hp(kt):
                hp_ps = ptail.tile([P, BL], dt.float32, tag="tailps",
                                   name=f"hp_ps{kt}")
                for ht in range(HT):
                    nc.tensor.matmul(hp_ps[:], waT_ap(kt, ht), htT_ap(ht),
                                     start=(ht == 0), stop=(ht == HT - 1))
                nc.vector.tensor_copy(hp_sb[:, kt, :], hp_ps[:])

            # ---- deferred post-op FIFO: one item emitted per main group ----
            post_q = []

            def pop_post():
                if post_q:
                    post_q.pop(0)()

            def make_epack(e_tile, kt, th0, th1):
                def emit():
                    for c in range(EC):
                        th = th0 if c < TC else th1
                        nc.tensor.matmul(
                            e_tile[32 * c:32 * c + 1, :EW],
                            va_ap(kt),
                            th[:, (c % TC) * EW:(c % TC + 1) * EW],
                            start=(kt == 0), stop=False,
                            tile_position=(0, 32 * c))
                return emit

            def make_mask_exp(e_tile, ex_row, ssum4, bi):
                def emit():
                    for c in range(EC):
                        nc.tensor.matmul(
                            e_tile[32 * c:32 * c + 1, :EW],
                            ones_b[:],
                            mask_m1[0:1, bi * T + c * EW:bi * T + (c + 1) * EW],
                            start=False, stop=True,
                            tile_position=(0, 32 * c))
                    # strip-exp straight out of PSUM into a [1, T] row
                    # (partition shift 32c -> 0), then per-chunk sums on DVE
                    # pipelined behind the ScalarE exps.
                    for c in range(EC):
                        nc.scalar.activation(
                            ex_row[:, c * EW:(c + 1) * EW],
                            e_tile[32 * c:32 * c + 1, :EW], AF.Exp)
                        nc.vector.tensor_reduce(
                            ssum4[:, c:c + 1], ex_row[:, c * EW:(c + 1) * EW],
                            axis=mybir.AxisListType.X, op=mybir.AluOpType.add)
                return emit

            def make_softmax(ex_row, ssum4, rinv, rinv_b):
                def emit():
                    ssum = smpool.tile([1, 1], dt.float32, tag="ssum", bufs=2)
                    nc.vector.tensor_reduce(ssum[:], ssum4[:],
                                            axis=mybir.AxisListType.X,
                                            op=mybir.AluOpType.add)
                    nc.vector.reciprocal(rinv[:], ssum[:])
                    nc.vector.tensor_copy(rinv_b[:], rinv[:])
                return emit

            def make_tail(bi, ex_row, rinv, rinv_b, nat_kc):
                def emit():
                    # transpose UNnormalized exp into partitions with 1/sum
                    # folded in via the rhs: atp[:, tt] = ex_chunk^T * rinv
                    atp = ptail.tile([P, TT], dt.float32, tag="tailps",
                                     name=f"atp{bi}")
                    for tt in range(TT):
                        nc.tensor.matmul(
                            atp[:, tt:tt + 1], ex_row[:, tt * P:(tt + 1) * P],
                            rinv_b[:], start=True, stop=True)
                    attnT = smpool.tile([P, TT], dt.bfloat16, tag="attnT",
                                        bufs=2)
                    nc.vector.tensor_copy(attnT[:], atp[:])
                    # attn output: ex * (1/sum), full fp32 row
                    attn_sb = smpool.tile([1, T], dt.float32, tag="attn",
                                          bufs=2)
                    nc.vector.tensor_scalar_mul(attn_sb[:], ex_row[:], rinv[:])
                    nc.scalar.dma_start(attn_d[bi:bi + 1, :], attn_sb[:])
                    # context: already normalized through attnT
                    cp = ptail.tile([P, NT], dt.float32, tag="tailps",
                                    name=f"cp{bi}")
                    for tt in range(TT):
                        for c in range(EC):
                            nc.tensor.matmul(
                                cp[32 * c:32 * c + 1, :EW],
                                attnT[:, tt:tt + 1],
                                nat_kc[c // TC][:, tt,
                                                (c % TC) * EW:(c % TC + 1) * EW],
                                start=(tt == 0), stop=(tt == TT - 1),
                                tile_position=(0, 32 * c))
                    ctx_sb = smpool.tile([1, H], dt.float32, tag="ctx", bufs=2)
                    for c in range(EC):
                        if c % 2 == 0:
                            nc.vector.tensor_copy(
                                ctx_sb[:, c * EW:(c + 1) * EW],
                                cp[32 * c:32 * c + 1, :EW])
                        else:
                            nc.scalar.copy(
                                ctx_sb[:, c * EW:(c + 1) * EW],
                                cp[32 * c:32 * c + 1, :EW])
                    nc.scalar.dma_start(ctx_d[bi:bi + 1, :], ctx_sb[:])
                return emit

            # ---- main loop: kt-outer for every batch ----
            for bi in range(BL):
                if bi == 0:
                    encT_t = encT_b0
                elif bi == 1:
                    encT_t = encT_b1
                else:
                    encT_t = encT_next
                e_tile = pe_ps.tile([P, NT], dt.float32, tag="e",
                                    name=f"e_ps{bi}")
                ex_row = smpool.tile([1, T], dt.bfloat16, tag="ex", bufs=2,
                                     name=f"ex{bi}")
                ssum4 = smpool.tile([1, EC], dt.float32, tag="ssum4", bufs=2,
                                    name=f"ssum4_{bi}")
                rinv = smpool.tile([1, 1], dt.float32, tag="rinv", bufs=2,
                                   name=f"rinv{bi}")
                rinv_b = smpool.tile([1, 1], dt.bfloat16, tag="rinv_b", bufs=2,
                                     name=f"rinvb{bi}")
                th_prev = None
                for gi, (kt, tcc) in enumerate(
                        [(kt, tcc) for kt in range(KT) for tcc in range(TC)]):
                    # prefetch emission points
                    if bi == 0:
                        if gi == 6:
                            encT_b1 = [load_encT(1, 0)]
                        elif gi == 10:
                            encT_b1.append(load_encT(1, 1))
                    else:
                        if gi == 2 and bi < BL - 1:
                            encT_next = [load_encT(bi + 1, 0)]
                        elif gi == 6 and bi < BL - 1:
                            encT_next.append(load_encT(bi + 1, 1))
                    if gi == 10:
                        nat_kc = [load_nat(bi, 0)]
                    elif gi == 12:
                        nat_kc.append(load_nat(bi, 1))
                    ps = pmain.tile([P, NT], dt.float32, tag="big")
                    for ht in range(HT):
                        nc.tensor.matmul(
                            ps[:], uaT_ap(kt, ht), encT_t[tcc][:, ht, :],
                            start=(ht == 0), stop=(ht == HT - 1))
                    if bi == 0 and tcc == 0:
                        emit_hp(kt)
                    th = thpool.tile([P, NT], dt.bfloat16, tag="th", name="th")
                    nc.scalar.activation(th[:], ps[:], AF.Tanh,
                                         bias=hp_sb[:, kt, bi:bi + 1])
                    pop_post()
                    if tcc == 1:
                        post_q.append(make_epack(e_tile, kt, th_prev, th))
                    th_prev = th
                post_q.append(make_mask_exp(e_tile, ex_row, ssum4, bi))
                post_q.append(make_softmax(ex_row, ssum4, rinv, rinv_b))
                post_q.append(make_tail(bi, ex_row, rinv, rinv_b, nat_kc))
            while post_q:
                post_q.pop(0)()

    nc.compile()
    return nc


def _get_runner():
    if "runner" in _CACHE:
        return _CACHE["runner"]

    import jax
    from jax.sharding import Mesh, PartitionSpec
    from jax.experimental.shard_map import shard_map
    from concourse import bass2jax
    from concourse import mybir as _mb

    nc = _build_nc()
    bass2jax.install_neuronx_cc_hook()

    partition_name = (nc.partition_id_tensor.name
                      if nc.partition_id_tensor else None)
    in_names, out_names, out_avals, zero_outs = [], [], [], []
    for alloc in nc.m.functions[0].allocations:
        if not isinstance(alloc, _mb.MemoryLocationSet):
            continue
        name = alloc.memorylocations[0].name
        if alloc.kind == "ExternalInput":
            if name != partition_name:
                in_names.append(name)
        elif alloc.kind == "ExternalOutput":
            out_names.append(name)
            shape = tuple(alloc.tensor_shape)
            npdt = _mb.dt.np(alloc.dtype)
            out_avals.append(jax.core.ShapedArray(shape, npdt))
            zero_outs.append(np.zeros(shape, npdt))
    n_params = len(in_names)
    n_outs = len(out_names)
    all_in_names = in_names + out_names
    if partition_name is not None:
        all_in_names = all_in_names + [partition_name]
    donate = tuple(range(n_params, n_params + n_outs))

    def _body(*args):
        operands = list(args)
        if partition_name is not None:
            operands.append(bass2jax.partition_id_tensor())
        outs = bass2jax._bass_exec_p.bind(
            *operands,
            out_avals=tuple(out_avals),
            in_names=tuple(all_in_names),
            out_names=tuple(out_names),
            lowering_input_output_aliases=(),
            sim_require_finite=True,
            sim_require_nnan=True,
            nc=nc,
        )
        return tuple(outs)

    devices = jax.devices()[:NCORES]
    mesh = Mesh(np.asarray(devices), ("core",))
    in_specs = (PartitionSpec("core"),) * (n_params + n_outs)
    out_specs = (PartitionSpec("core"),) * n_outs
    sharded = jax.jit(
        shard_map(_body, mesh=mesh, in_specs=in_specs, out_specs=out_specs,
                  check_rep=False),
        donate_argnums=donate, keep_unused=True)

    def run(in_maps):
        concat_in = [
            np.concatenate([np.asarray(m[name]) for m in in_maps], axis=0)
            for name in in_names
        ]
        concat_zeros = [
            np.zeros((NCORES * z.shape[0], *z.shape[1:]), z.dtype)
            for z in zero_outs
        ]
        out_arrs = sharded(*concat_in, *concat_zeros)
        return [
            {name: np.asarray(out_arrs[i]).reshape(NCORES, *out_avals[i].shape)[c]
             for i, name in enumerate(out_names)}
            for c in range(NCORES)
        ]

    _CACHE["runner"] = run
    return run


def _make_in_maps(inputs):
    import ml_dtypes
    bf16 = ml_dtypes.bfloat16

    h_t = np.asarray(inputs["h_t"], dtype=np.float32)
    enc_out = np.asarray(inputs["enc_out"], dtype=np.float32)
    src_mask = np.asarray(inputs["src_mask"])
    Wa = np.asarray(inputs["Wa"], dtype=np.float32)
    Ua = np.asarray(inputs["Ua"], dtype=np.float32)
    va = np.asarray(inputs["va"], dtype=np.float32)

    # [KT, P, HT, P] column blocks of Ua.T / Wa.T (lhsT layouts)
    uaT = np.ascontiguousarray(
        Ua.T.reshape(HT, P, KT, P).transpose(2, 1, 0, 3)).astype(bf16)
    waT = np.ascontiguousarray(
        Wa.T.reshape(HT, P, KT, P).transpose(2, 1, 0, 3)).astype(bf16)
    va_pk = np.ascontiguousarray(va.reshape(KT, P).T).astype(bf16)   # [P, KT]
    encT = np.ascontiguousarray(
        enc_out.transpose(0, 2, 1).reshape(B, HT, P, TC, NT)
        .transpose(0, 3, 2, 1, 4)).astype(bf16)                 # [B, TC, P, HT, NT]
    encn = np.ascontiguousarray(
        enc_out.reshape(B, TT, P, TC, NT)
        .transpose(0, 3, 2, 1, 4)).astype(bf16)                 # [B, TC, P, TT, NT]
    mask_u8 = np.ascontiguousarray(src_mask.astype(np.uint8))

    in_maps = []
    for c in range(NCORES):
        sl = slice(c * BL, (c + 1) * BL)
        htT = np.ascontiguousarray(
            h_t[sl].T.reshape(HT, P, BL).transpose(1, 0, 2)
            .reshape(P, HT * BL)).astype(bf16)                  # [P, HT*BL]
        wall = np.empty((P, WX), dtype=bf16)
        wall[:, OFF_UA0:OFF_VA] = uaT[0].reshape(P, HT * P)
        wall[:, OFF_VA:OFF_HTT] = va_pk
        wall[:, OFF_HTT:OFF_WA0] = htT
        wall[:, OFF_WA0:OFF_REST] = waT[0].reshape(P, HT * P)
        for kt in range(1, KT):
            o = OFF_REST + (kt - 1) * 2048
            wall[:, o:o + 1024] = uaT[kt].reshape(P, HT * P)
            wall[:, o + 1024:o + 2048] = waT[kt].reshape(P, HT * P)
        in_maps.append({
            "encT": encT[sl],
            "encn": encn[sl],
            "wall": wall,
            "mask": mask_u8[sl],
        })
    return in_maps


def kernel(h_t, enc_out, src_mask, Wa, Ua, va):
    in_maps = _make_in_maps({
        "h_t": h_t, "enc_out": enc_out, "src_mask": src_mask,
        "Wa": Wa, "Ua": Ua, "va": va,
    })
    run = _get_runner()
    results = run(in_maps)
    context = np.concatenate([r["ctx"] for r in results], axis=0)
    attn = np.concatenate([r["attn"] for r in results], axis=0)
    return context, attn


# revision 18
# speedup vs baseline: 1.6171x; 1.0638x over previous
"""Bahdanau additive attention on 8 Trainium2 NeuronCores.

Data-parallel over batch: core c handles batches [4c, 4c+4).
Per batch b:
  ep[k,t]   = sum_h Ua[k,h] * enc[b,t,h]        (bf16 PE matmuls, k on PSUM partitions)
  z[k,t]    = tanh(ep[k,t] + hp[b,k])           (ScalarE, hp as per-partition bias)
  e[t]      = sum_k va[k] * z[k,t]              (M=1 bf16 PE matmuls, 4x col-packed)
  attn      = softmax(e) * mask renorm          (strip-exp out of PSUM; no max-sub)
  ctx[h]    = sum_t attn[t] * enc[b,t,h]        (M=1 bf16 PE matmuls, 4x col-packed,
                                                 1/sum folded into the exp-transpose)
hp[b,k] = sum_h Wa[k,h] * h_t[b,h] runs as per-kt N=4 matmul chains whose tiny
weight chunks trickle in with the staged weight DMAs, filling the DMA-starved
prologue. All one-time weights (Ua blocks, Wa blocks, va, h_t^T) are packed into
a single [128, WX] DRAM tensor loaded with a handful of staged DMAs (descriptor
issue on the Sync engine costs ~0.7us per dma_start).
"""

import numpy as np

import concourse.bass as bass
import concourse.tile as tile
from concourse import bacc, mybir

dt = mybir.dt
AF = mybir.ActivationFunctionType

B, T, H = 32, 1024, 1024
NCORES = 8
BL = B // NCORES          # batches per core
P = 128                   # partitions
NT = 512                  # matmul free-dim chunk (one PSUM bank of fp32)
KT = H // P               # k-tiles (output rows of ep)
HT = H // P               # h-tiles (contraction)
TT = T // P               # t-tiles (partition tiles of natural enc)
TC = T // NT              # t chunks per batch
EC = 4                    # e/ctx col-packed chunks
EW = T // EC              # 256

# packed-weight column offsets (bf16 elements per partition)
OFF_UA0 = 0
OFF_VA = 1024
OFF_HTT = 1032
OFF_WA0 = 1064
OFF_REST = 2088           # kt>=1: [uaT_kt (1024) | waT_kt (1024)] blocks
WX = OFF_REST + (KT - 1) * 2048


def _off_ua(kt):
    return OFF_UA0 if kt == 0 else OFF_REST + (kt - 1) * 2048


def _off_wa(kt):
    return OFF_WA0 if kt == 0 else OFF_REST + (kt - 1) * 2048 + 1024


_CACHE = {}


def _build_nc():
    nc = bacc.Bacc("TRN2", target_bir_lowering=False, debug=False)

    encT_d = nc.dram_tensor("encT", [BL, TC, P, HT, NT], dt.bfloat16,
                            kind="ExternalInput").ap()
    encn_d = nc.dram_tensor("encn", [BL, TC, P, TT, NT], dt.bfloat16,
                            kind="ExternalInput").ap()
    wall_d = nc.dram_tensor("wall", [P, WX], dt.bfloat16,
                            kind="ExternalInput").ap()
    mask_d = nc.dram_tensor("mask", [BL, T], dt.uint8, kind="ExternalInput").ap()

    ctx_d = nc.dram_tensor("ctx", [BL, H], dt.float32, kind="ExternalOutput").ap()
    attn_d = nc.dram_tensor("attn", [BL, T], dt.float32, kind="ExternalOutput").ap()

    with tile.TileContext(nc) as tc:
        from contextlib import ExitStack

        with ExitStack() as st:
            wpool = st.enter_context(tc.tile_pool(name="weights", bufs=1))
            etpool = st.enter_context(tc.tile_pool(name="encT", bufs=4))
            natpool = st.enter_context(tc.tile_pool(name="nat", bufs=4))
            thpool = st.enter_context(tc.tile_pool(name="tanh", bufs=4))
            smpool = st.enter_context(tc.tile_pool(name="small", bufs=1))
            pmain = st.enter_context(tc.tile_pool(name="pmain", bufs=4, space="PSUM"))
            pe_ps = st.enter_context(tc.tile_pool(name="pe", bufs=2, space="PSUM"))
            ptail = st.enter_context(tc.tile_pool(name="ptail", bufs=2, space="PSUM"))

            wall_sb = wpool.tile([P, WX], dt.bfloat16, tag="wall")

            def uaT_ap(kt, ht):
                o = _off_ua(kt) + ht * P
                return wall_sb[:, o:o + P]

            def waT_ap(kt, ht):
                o = _off_wa(kt) + ht * P
                return wall_sb[:, o:o + P]

            def htT_ap(ht):
                o = OFF_HTT + ht * BL
                return wall_sb[:, o:o + BL]

            def va_ap(kt):
                return wall_sb[:, OFF_VA + kt:OFF_VA + kt + 1]

            def load_wall(c0, c1):
                nc.sync.dma_start(wall_sb[:, c0:c1], wall_d[:, c0:c1])

            def load_encT(bi, tcc, eng, split=False):
                t_ = etpool.tile([P, HT, NT], dt.bfloat16, tag="encT",
                                 name=f"encT{bi}_{tcc}")
                if split:
                    # alternate halves across the two HWDGE queues so more
                    # DMA engines engage concurrently in the prologue
                    nc.sync.dma_start(t_[:, 0:2, :], encT_d[bi, tcc, :, 0:2, :])
                    nc.scalar.dma_start(t_[:, 2:4, :], encT_d[bi, tcc, :, 2:4, :])
                    nc.sync.dma_start(t_[:, 4:6, :], encT_d[bi, tcc, :, 4:6, :])
                    nc.scalar.dma_start(t_[:, 6:8, :], encT_d[bi, tcc, :, 6:8, :])
                else:
                    eng.dma_start(t_[:], encT_d[bi, tcc])
                return t_

            def load_nat(bi, kc):
                t_ = natpool.tile([P, TT, NT], dt.bfloat16, tag="nat",
                                  name=f"nat{bi}_{kc}")
                nc.scalar.dma_start(t_[:], encn_d[bi, kc])
                return t_

            def load_wall2(c0, c1, eng):
                eng.dma_start(wall_sb[:, c0:c1], wall_d[:, c0:c1])

            # prologue DMAs, need-ordered, striped across both queues
            load_wall2(OFF_UA0, OFF_UA0 + 512, nc.sync)       # uaT0 a
            load_wall2(OFF_UA0 + 512, OFF_VA, nc.scalar)      # uaT0 b
            encT_b0 = [load_encT(0, 0, None, split=True)]
            load_wall2(OFF_VA, OFF_WA0, nc.sync)              # va + htT
            load_wall2(OFF_WA0, OFF_REST, nc.scalar)          # waT0
            load_wall2(_off_ua(1), _off_ua(2), nc.sync)       # kt1
            load_wall2(_off_ua(2), _off_ua(3), nc.scalar)     # kt2
            load_wall2(_off_ua(3), _off_ua(4), nc.sync)       # kt3
            load_wall2(_off_ua(4), _off_ua(5), nc.scalar)     # kt4
            load_wall2(_off_ua(5), _off_ua(6), nc.sync)       # kt5
            load_wall2(_off_ua(6), _off_ua(7), nc.scalar)     # kt6
            load_wall2(_off_ua(7), WX, nc.sync)               # kt7

            ones_b = wpool.tile([1, 1], dt.bfloat16, tag="ones_b")
            nc.vector.memset(ones_b[:], 1.0)
            hp_sb = wpool.tile([P, KT, BL], dt.float32, tag="hp")

            # mask for all batches: cast-DMA + one dual-op DVE pass (DVE is
            # idle in the prologue; ScalarE is not): mask_m1 = (mask-1)*1e30
            mask_f = smpool.tile([1, BL * T], dt.float32, tag="mask")
            nc.gpsimd.dma_start(mask_f[:], mask_d.rearrange("b t -> (b t)"))
            mask_m1 = smpool.tile([1, BL * T], dt.bfloat16, tag="mask_m1")
            nc.vector.tensor_scalar(mask_m1[:], mask_f[:], 1.0, 1e30,
                                    mybir.AluOpType.subtract,
                                    mybir.AluOpType.mult)

            # PE warm-up: back-to-back dummy matmuls while the prologue DMAs
            # stream in, so HAM reaches K=8/8 before the first real group
            g_rhs = wpool.tile([P, NT], dt.bfloat16, tag="g_rhs")
            nc.vector.memset(g_rhs[:], 0.0)
            warm_ps = ptail.tile([P, NT], dt.float32, tag="tailps",
                                 name="warm_ps")
            for _ in range(16):
                nc.tensor.matmul(warm_ps[:], g_rhs[:, :P], g_rhs[:],
                                 start=True, stop=True)
            encT_b0.append(load_encT(0, 1, None, split=True))

            def emit_hp(kt):
                hp_ps = ptail.tile([P, BL], dt.float32, tag="tailps",
                                   name=f"hp_ps{kt}")
                for ht in range(HT):
                    nc.tensor.matmul(hp_ps[:], waT_ap(kt, ht), htT_ap(ht),
                                     start=(ht == 0), stop=(ht == HT - 1))
                nc.vector.tensor_copy(hp_sb[:, kt, :], hp_ps[:])

            # ---- deferred post-op FIFO: one item emitted per main group ----
            post_q = []

            def pop_post():
                if post_q:
                    post_q.pop(0)()

            def make_epack(e_tile, kt, th0, th1):
                def emit():
                    for c in range(EC):
                        th = th0 if c < TC else th1
                        nc.tensor.matmul(
                            e_tile[32 * c:32 * c + 1, :EW],
                            va_ap(kt),
                            th[:, (c % TC) * EW:(c % TC + 1) * EW],
                            start=(kt == 0), stop=False,
                            tile_position=(0, 32 * c))
                return emit

            def make_mask_exp(e_tile, ex_row, ssum4, atp_box, bi, inline_atp):
                def emit():
                    atp = ptail.tile([P, TT], dt.float32, tag="tailps",
                                     name=f"atp{bi}")
                    atp_box.append(atp)
                    for c in range(EC):
                        nc.tensor.matmul(
                            e_tile[32 * c:32 * c + 1, :EW],
                            ones_b[:],
                            mask_m1[0:1, bi * T + c * EW:bi * T + (c + 1) * EW],
                            start=False, stop=True,
                            tile_position=(0, 32 * c))
                    # strip-exp straight out of PSUM into a [1, T] row
                    # (partition shift 32c -> 0), then per-chunk sums on DVE
                    # pipelined behind the ScalarE exps.
                    for c in range(EC):
                        nc.scalar.activation(
                            ex_row[:, c * EW:(c + 1) * EW],
                            e_tile[32 * c:32 * c + 1, :EW], AF.Exp)
                        nc.vector.tensor_reduce(
                            ssum4[:, c:c + 1], ex_row[:, c * EW:(c + 1) * EW],
                            axis=mybir.AxisListType.X, op=mybir.AluOpType.add)
                        if inline_atp:
                            for tt in (2 * c, 2 * c + 1):
                                nc.tensor.matmul(
                                    atp[:, tt:tt + 1],
                                    ex_row[:, tt * P:(tt + 1) * P],
                                    ones_b[:], start=True, stop=True)
                return emit

            def make_softmax(ssum4, rinv):
                def emit():
                    ssum = smpool.tile([1, 1], dt.float32, tag="ssum", bufs=2)
                    nc.vector.tensor_reduce(ssum[:], ssum4[:],
                                            axis=mybir.AxisListType.X,
                                            op=mybir.AluOpType.add)
                    nc.vector.reciprocal(rinv[:], ssum[:])
                return emit

            def make_tail(bi, ex_row, rinv, atp_box, nat_kc, inline_atp):
                def emit():
                    # transpose UNnormalized exp into partitions: 1/sum is
                    # applied later on the ctx strips, so this does not wait
                    # for the softmax sum.
                    atp = atp_box[0]
                    if not inline_atp:
                        for tt in range(TT):
                            nc.tensor.matmul(
                                atp[:, tt:tt + 1],
                                ex_row[:, tt * P:(tt + 1) * P],
                                ones_b[:], start=True, stop=True)
                    attnT = smpool.tile([P, TT], dt.bfloat16, tag="attnT",
                                        bufs=2)
                    nc.vector.tensor_copy(attnT[:], atp[:])
                    # attn output: ex * (1/sum), full fp32 row
                    attn_sb = smpool.tile([1, T], dt.float32, tag="attn",
                                          bufs=2)
                    nc.vector.tensor_scalar_mul(attn_sb[:], ex_row[:], rinv[:])
                    nc.scalar.dma_start(attn_d[bi:bi + 1, :], attn_sb[:])
                    # context: normalize while draining the PSUM strips
                    cp = ptail.tile([P, NT], dt.float32, tag="tailps",
                                    name=f"cp{bi}")
                    for tt in range(TT):
                        for c in range(EC):
                            nc.tensor.matmul(
                                cp[32 * c:32 * c + 1, :EW],
                                attnT[:, tt:tt + 1],
                                nat_kc[c // TC][:, tt,
                                                (c % TC) * EW:(c % TC + 1) * EW],
                                start=(tt == 0), stop=(tt == TT - 1),
                                tile_position=(0, 32 * c))
                    ctx_sb = smpool.tile([1, H], dt.float32, tag="ctx", bufs=2)
                    for c in range(EC):
                        if c % 2 == 0:
                            nc.vector.tensor_scalar_mul(
                                ctx_sb[:, c * EW:(c + 1) * EW],
                                cp[32 * c:32 * c + 1, :EW], rinv[:])
                        else:
                            nc.scalar.mul(
                                ctx_sb[:, c * EW:(c + 1) * EW],
                                cp[32 * c:32 * c + 1, :EW], rinv[:])
                    nc.scalar.dma_start(ctx_d[bi:bi + 1, :], ctx_sb[:])
                return emit

            # ---- main loop: tcc-outer for batch 0 (DMA need-order),
            # kt-outer for the rest (both encT tiles prefetched) ----
            for bi in range(BL):
                if bi == 0:
                    encT_t = encT_b0
                    group_iter = [(kt, tcc) for tcc in range(TC)
                                  for kt in range(KT)]
                elif bi == 1:
                    encT_t = encT_b1
                else:
                    encT_t = encT_next
                if bi > 0:
                    group_iter = [(kt, tcc) for kt in range(KT)
                                  for tcc in range(TC)]
                e_tile = pe_ps.tile([P, NT], dt.float32, tag="e",
                                    name=f"e_ps{bi}")
                ex_row = smpool.tile([1, T], dt.bfloat16, tag="ex", bufs=2,
                                     name=f"ex{bi}")
                ssum4 = smpool.tile([1, EC], dt.float32, tag="ssum4", bufs=2,
                                    name=f"ssum4_{bi}")
                rinv = smpool.tile([1, 1], dt.float32, tag="rinv", bufs=2,
                                   name=f"rinv{bi}")
                atp_box = []
                th0_of = {}
                for gi, (kt, tcc) in enumerate(group_iter):
                    # prefetch emission points
                    if bi == 0:
                        if gi == 10:
                            encT_b1 = [load_encT(1, 0, nc.sync)]
                            nat_kc = [load_nat(bi, 0)]
                        elif gi == 12:
                            encT_b1.append(load_encT(1, 1, nc.sync))
                            nat_kc.append(load_nat(bi, 1))
                    else:
                        if gi == 2 and bi < BL - 1:
                            encT_next = [load_encT(bi + 1, 0, nc.sync)]
                        elif gi == 6 and bi < BL - 1:
                            encT_next.append(load_encT(bi + 1, 1, nc.sync))
                        if gi == 10:
                            nat_kc = [load_nat(bi, 0)]
                        elif gi == 12:
                            nat_kc.append(load_nat(bi, 1))
                    ps = pmain.tile([P, NT], dt.float32, tag="big")
                    for ht in range(HT):
                        nc.tensor.matmul(
                            ps[:], uaT_ap(kt, ht), encT_t[tcc][:, ht, :],
                            start=(ht == 0), stop=(ht == HT - 1))
                    if bi == 0 and tcc == 0:
                        emit_hp(kt)
                    th = thpool.tile([P, NT], dt.bfloat16, tag="th",
                                     bufs=12, name="th")
                    nc.scalar.activation(th[:], ps[:], AF.Tanh,
                                         bias=hp_sb[:, kt, bi:bi + 1])
                    pop_post()
                    if tcc == 0:
                        th0_of[kt] = th
                    else:
                        post_q.append(make_epack(e_tile, kt, th0_of[kt], th))
                inline_atp = (bi == BL - 1)
                post_q.append(make_mask_exp(e_tile, ex_row, ssum4, atp_box,
                                            bi, inline_atp))
                post_q.append(make_softmax(ssum4, rinv))
                post_q.append(make_tail(bi, ex_row, rinv, atp_box, nat_kc,
                                        inline_atp))
            while post_q:
                post_q.pop(0)()

    nc.compile()
    return nc


def _get_runner():
    if "runner" in _CACHE:
        return _CACHE["runner"]

    import jax
    from jax.sharding import Mesh, PartitionSpec
    from jax.experimental.shard_map import shard_map
    from concourse import bass2jax
    from concourse import mybir as _mb

    nc = _build_nc()
    bass2jax.install_neuronx_cc_hook()

    partition_name = (nc.partition_id_tensor.name
                      if nc.partition_id_tensor else None)
    in_names, out_names, out_avals, zero_outs = [], [], [], []
    for alloc in nc.m.functions[0].allocations:
        if not isinstance(alloc, _mb.MemoryLocationSet):
            continue
        name = alloc.memorylocations[0].name
        if alloc.kind == "ExternalInput":
            if name != partition_name:
                in_names.append(name)
        elif alloc.kind == "ExternalOutput":
            out_names.append(name)
            shape = tuple(alloc.tensor_shape)
            npdt = _mb.dt.np(alloc.dtype)
            out_avals.append(jax.core.ShapedArray(shape, npdt))
            zero_outs.append(np.zeros(shape, npdt))
    n_params = len(in_names)
    n_outs = len(out_names)
    all_in_names = in_names + out_names
    if partition_name is not None:
        all_in_names = all_in_names + [partition_name]
    donate = tuple(range(n_params, n_params + n_outs))

    def _body(*args):
        operands = list(args)
        if partition_name is not None:
            operands.append(bass2jax.partition_id_tensor())
        outs = bass2jax._bass_exec_p.bind(
            *operands,
            out_avals=tuple(out_avals),
            in_names=tuple(all_in_names),
            out_names=tuple(out_names),
            lowering_input_output_aliases=(),
            sim_require_finite=True,
            sim_require_nnan=True,
            nc=nc,
        )
        return tuple(outs)

    devices = jax.devices()[:NCORES]
    mesh = Mesh(np.asarray(devices), ("core",))
    in_specs = (PartitionSpec("core"),) * (n_params + n_outs)
    out_specs = (PartitionSpec("core"),) * n_outs
    sharded = jax.jit(
        shard_map(_body, mesh=mesh, in_specs=in_specs, out_specs=out_specs,
                  check_rep=False),
        donate_argnums=donate, keep_unused=True)

    def run(in_maps):
        concat_in = [
            np.concatenate([np.asarray(m[name]) for m in in_maps], axis=0)
            for name in in_names
        ]
        concat_zeros = [
            np.zeros((NCORES * z.shape[0], *z.shape[1:]), z.dtype)
            for z in zero_outs
        ]
        out_arrs = sharded(*concat_in, *concat_zeros)
        return [
            {name: np.asarray(out_arrs[i]).reshape(NCORES, *out_avals[i].shape)[c]
             for i, name in enumerate(out_names)}
            for c in range(NCORES)
        ]

    _CACHE["runner"] = run
    return run


def _make_in_maps(inputs):
    import ml_dtypes
    bf16 = ml_dtypes.bfloat16

    h_t = np.asarray(inputs["h_t"], dtype=np.float32)
    enc_out = np.asarray(inputs["enc_out"], dtype=np.float32)
    src_mask = np.asarray(inputs["src_mask"])
    Wa = np.asarray(inputs["Wa"], dtype=np.float32)
    Ua = np.asarray(inputs["Ua"], dtype=np.float32)
    va = np.asarray(inputs["va"], dtype=np.float32)

    # [KT, P, HT, P] column blocks of Ua.T / Wa.T (lhsT layouts)
    uaT = np.ascontiguousarray(
        Ua.T.reshape(HT, P, KT, P).transpose(2, 1, 0, 3)).astype(bf16)
    waT = np.ascontiguousarray(
        Wa.T.reshape(HT, P, KT, P).transpose(2, 1, 0, 3)).astype(bf16)
    va_pk = np.ascontiguousarray(va.reshape(KT, P).T).astype(bf16)   # [P, KT]
    encT = np.ascontiguousarray(
        enc_out.transpose(0, 2, 1).reshape(B, HT, P, TC, NT)
        .transpose(0, 3, 2, 1, 4)).astype(bf16)                 # [B, TC, P, HT, NT]
    encn = np.ascontiguousarray(
        enc_out.reshape(B, TT, P, TC, NT)
        .transpose(0, 3, 2, 1, 4)).astype(bf16)                 # [B, TC, P, TT, NT]
    mask_u8 = np.ascontiguousarray(src_mask.astype(np.uint8))

    in_maps = []
    for c in range(NCORES):
        sl = slice(c * BL, (c + 1) * BL)
        htT = np.ascontiguousarray(
            h_t[sl].T.reshape(HT, P, BL).transpose(1, 0, 2)
            .reshape(P, HT * BL)).astype(bf16)                  # [P, HT*BL]
        wall = np.empty((P, WX), dtype=bf16)
        wall[:, OFF_UA0:OFF_VA] = uaT[0].reshape(P, HT * P)
        wall[:, OFF_VA:OFF_HTT] = va_pk
        wall[:, OFF_HTT:OFF_WA0] = htT
        wall[:, OFF_WA0:OFF_REST] = waT[0].reshape(P, HT * P)
        for kt in range(1, KT):
            o = OFF_REST + (kt - 1) * 2048
            wall[:, o:o + 1024] = uaT[kt].reshape(P, HT * P)
            wall[:, o + 1024:o + 2048] = waT[kt].reshape(P, HT * P)
        in_maps.append({
            "encT": encT[sl],
            "encn": encn[sl],
            "wall": wall,
            "mask": mask_u8[sl],
        })
    return in_maps


def kernel(h_t, enc_out, src_mask, Wa, Ua, va):
    in_maps = _make_in_maps({
        "h_t": h_t, "enc_out": enc_out, "src_mask": src_mask,
        "Wa": Wa, "Ua": Ua, "va": va,
    })
    run = _get_runner()
    results = run(in_maps)
    context = np.concatenate([r["ctx"] for r in results], axis=0)
    attn = np.concatenate([r["attn"] for r in results], axis=0)
    return context, attn


# revision 19
# speedup vs baseline: 1.6282x; 1.0069x over previous
"""Bahdanau additive attention on 8 Trainium2 NeuronCores.

Data-parallel over batch: core c handles batches [4c, 4c+4).
Per batch b:
  ep[k,t]   = sum_h Ua[k,h] * enc[b,t,h]        (bf16 PE matmuls, k on PSUM partitions)
  z[k,t]    = tanh(ep[k,t] + hp[b,k])           (ScalarE, hp as per-partition bias)
  e[t]      = sum_k va[k] * z[k,t]              (M=1 bf16 PE matmuls, 4x col-packed)
  attn      = softmax(e) * mask renorm          (strip-exp out of PSUM; no max-sub)
  ctx[h]    = sum_t attn[t] * enc[b,t,h]        (M=1 bf16 PE matmuls, 4x col-packed,
                                                 1/sum folded into the exp-transpose)
hp[b,k] = sum_h Wa[k,h] * h_t[b,h] runs as per-kt N=4 matmul chains whose tiny
weight chunks trickle in with the staged weight DMAs, filling the DMA-starved
prologue. All one-time weights (Ua blocks, Wa blocks, va, h_t^T) are packed into
a single [128, WX] DRAM tensor loaded with a handful of staged DMAs (descriptor
issue on the Sync engine costs ~0.7us per dma_start).
"""

import numpy as np

import concourse.bass as bass
import concourse.tile as tile
from concourse import bacc, mybir

dt = mybir.dt
AF = mybir.ActivationFunctionType

B, T, H = 32, 1024, 1024
NCORES = 8
BL = B // NCORES          # batches per core
P = 128                   # partitions
NT = 512                  # matmul free-dim chunk (one PSUM bank of fp32)
KT = H // P               # k-tiles (output rows of ep)
HT = H // P               # h-tiles (contraction)
TT = T // P               # t-tiles (partition tiles of natural enc)
TC = T // NT              # t chunks per batch
EC = 4                    # e/ctx col-packed chunks
EW = T // EC              # 256

# packed-weight column offsets (bf16 elements per partition)
OFF_UA0 = 0
OFF_VA = 1024
OFF_HTT = 1032
OFF_WA0 = 1064
OFF_REST = 2088           # kt>=1: [uaT_kt (1024) | waT_kt (1024)] blocks
WX = OFF_REST + (KT - 1) * 2048


def _off_ua(kt):
    return OFF_UA0 if kt == 0 else OFF_REST + (kt - 1) * 2048


def _off_wa(kt):
    return OFF_WA0 if kt == 0 else OFF_REST + (kt - 1) * 2048 + 1024


_CACHE = {}


def _build_nc():
    nc = bacc.Bacc("TRN2", target_bir_lowering=False, debug=False)

    encT_d = nc.dram_tensor("encT", [BL, TC, P, HT, NT], dt.bfloat16,
                            kind="ExternalInput").ap()
    encn_d = nc.dram_tensor("encn", [BL, TC, P, TT, NT], dt.bfloat16,
                            kind="ExternalInput").ap()
    wall_d = nc.dram_tensor("wall", [P, WX], dt.bfloat16,
                            kind="ExternalInput").ap()
    mask_d = nc.dram_tensor("mask", [BL, T], dt.uint8, kind="ExternalInput").ap()

    ctx_d = nc.dram_tensor("ctx", [BL, H], dt.float32, kind="ExternalOutput").ap()
    attn_d = nc.dram_tensor("attn", [BL, T], dt.float32, kind="ExternalOutput").ap()

    with tile.TileContext(nc) as tc:
        from contextlib import ExitStack

        with ExitStack() as st:
            wpool = st.enter_context(tc.tile_pool(name="weights", bufs=1))
            etpool = st.enter_context(tc.tile_pool(name="encT", bufs=4))
            natpool = st.enter_context(tc.tile_pool(name="nat", bufs=4))
            thpool = st.enter_context(tc.tile_pool(name="tanh", bufs=4))
            smpool = st.enter_context(tc.tile_pool(name="small", bufs=1))
            pmain = st.enter_context(tc.tile_pool(name="pmain", bufs=4, space="PSUM"))
            pe_ps = st.enter_context(tc.tile_pool(name="pe", bufs=2, space="PSUM"))
            ptail = st.enter_context(tc.tile_pool(name="ptail", bufs=2, space="PSUM"))

            wall_sb = wpool.tile([P, WX], dt.bfloat16, tag="wall")

            def uaT_ap(kt, ht):
                o = _off_ua(kt) + ht * P
                return wall_sb[:, o:o + P]

            def waT_ap(kt, ht):
                o = _off_wa(kt) + ht * P
                return wall_sb[:, o:o + P]

            def htT_ap(ht):
                o = OFF_HTT + ht * BL
                return wall_sb[:, o:o + BL]

            def va_ap(kt):
                return wall_sb[:, OFF_VA + kt:OFF_VA + kt + 1]

            def load_wall(c0, c1):
                nc.sync.dma_start(wall_sb[:, c0:c1], wall_d[:, c0:c1])

            def load_encT(bi, tcc, eng, split=False):
                t_ = etpool.tile([P, HT, NT], dt.bfloat16, tag="encT",
                                 name=f"encT{bi}_{tcc}")
                if split:
                    # alternate halves across the two HWDGE queues so more
                    # DMA engines engage concurrently in the prologue
                    nc.sync.dma_start(t_[:, 0:2, :], encT_d[bi, tcc, :, 0:2, :])
                    nc.scalar.dma_start(t_[:, 2:4, :], encT_d[bi, tcc, :, 2:4, :])
                    nc.sync.dma_start(t_[:, 4:6, :], encT_d[bi, tcc, :, 4:6, :])
                    nc.scalar.dma_start(t_[:, 6:8, :], encT_d[bi, tcc, :, 6:8, :])
                else:
                    eng.dma_start(t_[:], encT_d[bi, tcc])
                return t_

            def load_nat(bi, kc):
                t_ = natpool.tile([P, TT, NT], dt.bfloat16, tag="nat",
                                  name=f"nat{bi}_{kc}")
                nc.scalar.dma_start(t_[:], encn_d[bi, kc])
                return t_

            def load_wall2(c0, c1, eng):
                eng.dma_start(wall_sb[:, c0:c1], wall_d[:, c0:c1])

            # prologue DMAs, need-ordered, striped across both queues:
            # per kt, Ua block rides the sync queue and Wa block the scalar
            # queue so both queues advance one half-block per group.
            load_wall2(OFF_UA0, OFF_UA0 + 512, nc.sync)       # uaT0 a
            load_wall2(OFF_UA0 + 512, OFF_VA, nc.scalar)      # uaT0 b
            load_wall2(OFF_VA, OFF_WA0, nc.sync)              # va + htT (tiny)
            encT_b0 = [load_encT(0, 0, None, split=True)]
            load_wall2(OFF_WA0, OFF_WA0 + 512, nc.sync)       # waT0 a
            load_wall2(OFF_WA0 + 512, OFF_REST, nc.scalar)    # waT0 b
            for _kt in range(1, KT):
                load_wall2(_off_ua(_kt), _off_wa(_kt), nc.sync)    # uaT_kt
                load_wall2(_off_wa(_kt), _off_ua(_kt + 1) if _kt < KT - 1
                           else WX, nc.scalar)                     # waT_kt

            ones_b = wpool.tile([1, 1], dt.bfloat16, tag="ones_b")
            nc.vector.memset(ones_b[:], 1.0)
            hp_sb = wpool.tile([P, KT, BL], dt.float32, tag="hp")

            # mask for all batches: cast-DMA + one dual-op DVE pass (DVE is
            # idle in the prologue; ScalarE is not): mask_m1 = (mask-1)*1e30
            mask_f = smpool.tile([1, BL * T], dt.float32, tag="mask")
            nc.gpsimd.dma_start(mask_f[:], mask_d.rearrange("b t -> (b t)"))
            mask_m1 = smpool.tile([1, BL * T], dt.bfloat16, tag="mask_m1")
            nc.vector.tensor_scalar(mask_m1[:], mask_f[:], 1.0, 1e30,
                                    mybir.AluOpType.subtract,
                                    mybir.AluOpType.mult)

            # PE warm-up: back-to-back dummy matmuls while the prologue DMAs
            # stream in, so HAM reaches K=8/8 before the first real group
            g_rhs = wpool.tile([P, NT], dt.bfloat16, tag="g_rhs")
            nc.vector.memset(g_rhs[:], 0.0)
            warm_ps = ptail.tile([P, NT], dt.float32, tag="tailps",
                                 name="warm_ps")
            for _ in range(16):
                nc.tensor.matmul(warm_ps[:], g_rhs[:, :P], g_rhs[:],
                                 start=True, stop=True)
            encT_b0.append(load_encT(0, 1, None, split=True))

            def emit_hp(kt):
                hp_ps = ptail.tile([P, BL], dt.float32, tag="tailps",
                                   name=f"hp_ps{kt}")
                for ht in range(HT):
                    nc.tensor.matmul(hp_ps[:], waT_ap(kt, ht), htT_ap(ht),
                                     start=(ht == 0), stop=(ht == HT - 1))
                nc.vector.tensor_copy(hp_sb[:, kt, :], hp_ps[:])

            # ---- deferred post-op FIFO: one item emitted per main group ----
            post_q = []

            def pop_post():
                if post_q:
                    post_q.pop(0)()

            def make_epack(e_tile, kt, th0, th1):
                def emit():
                    for c in range(EC):
                        th = th0 if c < TC else th1
                        nc.tensor.matmul(
                            e_tile[32 * c:32 * c + 1, :EW],
                            va_ap(kt),
                            th[:, (c % TC) * EW:(c % TC + 1) * EW],
                            start=(kt == 0), stop=False,
                            tile_position=(0, 32 * c))
                return emit

            def make_mask_exp(e_tile, ex_row, ssum4, atp_box, bi, inline_atp):
                def emit():
                    atp = ptail.tile([P, TT], dt.float32, tag="tailps",
                                     name=f"atp{bi}")
                    atp_box.append(atp)
                    for c in range(EC):
                        nc.tensor.matmul(
                            e_tile[32 * c:32 * c + 1, :EW],
                            ones_b[:],
                            mask_m1[0:1, bi * T + c * EW:bi * T + (c + 1) * EW],
                            start=False, stop=True,
                            tile_position=(0, 32 * c))
                    # strip-exp straight out of PSUM into a [1, T] row
                    # (partition shift 32c -> 0), then per-chunk sums on DVE
                    # pipelined behind the ScalarE exps.
                    for c in range(EC):
                        nc.scalar.activation(
                            ex_row[:, c * EW:(c + 1) * EW],
                            e_tile[32 * c:32 * c + 1, :EW], AF.Exp)
                        nc.vector.tensor_reduce(
                            ssum4[:, c:c + 1], ex_row[:, c * EW:(c + 1) * EW],
                            axis=mybir.AxisListType.X, op=mybir.AluOpType.add)
                        if inline_atp:
                            for tt in (2 * c, 2 * c + 1):
                                nc.tensor.matmul(
                                    atp[:, tt:tt + 1],
                                    ex_row[:, tt * P:(tt + 1) * P],
                                    ones_b[:], start=True, stop=True)
                return emit

            def make_softmax(ssum4, rinv):
                def emit():
                    ssum = smpool.tile([1, 1], dt.float32, tag="ssum", bufs=2)
                    nc.vector.tensor_reduce(ssum[:], ssum4[:],
                                            axis=mybir.AxisListType.X,
                                            op=mybir.AluOpType.add)
                    nc.vector.reciprocal(rinv[:], ssum[:])
                return emit

            def make_tail(bi, ex_row, rinv, atp_box, nat_kc, inline_atp):
                def emit():
                    # transpose UNnormalized exp into partitions: 1/sum is
                    # applied later on the ctx strips, so this does not wait
                    # for the softmax sum.
                    atp = atp_box[0]
                    if not inline_atp:
                        for tt in range(TT):
                            nc.tensor.matmul(
                                atp[:, tt:tt + 1],
                                ex_row[:, tt * P:(tt + 1) * P],
                                ones_b[:], start=True, stop=True)
                    attnT = smpool.tile([P, TT], dt.bfloat16, tag="attnT",
                                        bufs=2)
                    nc.vector.tensor_copy(attnT[:], atp[:])
                    # attn output: ex * (1/sum), full fp32 row
                    attn_sb = smpool.tile([1, T], dt.float32, tag="attn",
                                          bufs=2)
                    nc.vector.tensor_scalar_mul(attn_sb[:], ex_row[:], rinv[:])
                    nc.scalar.dma_start(attn_d[bi:bi + 1, :], attn_sb[:])
                    # context: normalize while draining the PSUM strips
                    cp = ptail.tile([P, NT], dt.float32, tag="tailps",
                                    name=f"cp{bi}")
                    for tt in range(TT):
                        for c in range(EC):
                            nc.tensor.matmul(
                                cp[32 * c:32 * c + 1, :EW],
                                attnT[:, tt:tt + 1],
                                nat_kc[c // TC][:, tt,
                                                (c % TC) * EW:(c % TC + 1) * EW],
                                start=(tt == 0), stop=(tt == TT - 1),
                                tile_position=(0, 32 * c))
                    ctx_sb = smpool.tile([1, H], dt.float32, tag="ctx", bufs=2)
                    for c in range(EC):
                        if c % 2 == 0:
                            nc.vector.tensor_scalar_mul(
                                ctx_sb[:, c * EW:(c + 1) * EW],
                                cp[32 * c:32 * c + 1, :EW], rinv[:])
                        else:
                            nc.scalar.mul(
                                ctx_sb[:, c * EW:(c + 1) * EW],
                                cp[32 * c:32 * c + 1, :EW], rinv[:])
                    nc.scalar.dma_start(ctx_d[bi:bi + 1, :], ctx_sb[:])
                return emit

            # ---- main loop: tcc-outer for batch 0 (DMA need-order),
            # kt-outer for the rest (both encT tiles prefetched) ----
            for bi in range(BL):
                if bi == 0:
                    encT_t = encT_b0
                    group_iter = [(kt, tcc) for tcc in range(TC)
                                  for kt in range(KT)]
                elif bi == 1:
                    encT_t = encT_b1
                else:
                    encT_t = encT_next
                if bi > 0:
                    group_iter = [(kt, tcc) for kt in range(KT)
                                  for tcc in range(TC)]
                e_tile = pe_ps.tile([P, NT], dt.float32, tag="e",
                                    name=f"e_ps{bi}")
                ex_row = smpool.tile([1, T], dt.bfloat16, tag="ex", bufs=2,
                                     name=f"ex{bi}")
                ssum4 = smpool.tile([1, EC], dt.float32, tag="ssum4", bufs=2,
                                    name=f"ssum4_{bi}")
                rinv = smpool.tile([1, 1], dt.float32, tag="rinv", bufs=2,
                                   name=f"rinv{bi}")
                atp_box = []
                th0_of = {}
                for gi, (kt, tcc) in enumerate(group_iter):
                    # prefetch emission points
                    if bi == 0:
                        if gi == 10:
                            encT_b1 = [load_encT(1, 0, nc.sync)]
                            nat_kc = [load_nat(bi, 0)]
                        elif gi == 12:
                            encT_b1.append(load_encT(1, 1, nc.sync))
                            nat_kc.append(load_nat(bi, 1))
                    else:
                        if gi == 2 and bi < BL - 1:
                            encT_next = [load_encT(bi + 1, 0, nc.sync)]
                        elif gi == 6 and bi < BL - 1:
                            encT_next.append(load_encT(bi + 1, 1, nc.sync))
                        if gi == 10:
                            nat_kc = [load_nat(bi, 0)]
                        elif gi == 12:
                            nat_kc.append(load_nat(bi, 1))
                    ps = pmain.tile([P, NT], dt.float32, tag="big")
                    for ht in range(HT):
                        nc.tensor.matmul(
                            ps[:], uaT_ap(kt, ht), encT_t[tcc][:, ht, :],
                            start=(ht == 0), stop=(ht == HT - 1))
                    if bi == 0 and tcc == 0:
                        emit_hp(kt)
                    th = thpool.tile([P, NT], dt.bfloat16, tag="th",
                                     bufs=12, name="th")
                    nc.scalar.activation(th[:], ps[:], AF.Tanh,
                                         bias=hp_sb[:, kt, bi:bi + 1])
                    pop_post()
                    if tcc == 0:
                        th0_of[kt] = th
                    else:
                        post_q.append(make_epack(e_tile, kt, th0_of[kt], th))
                inline_atp = (bi == BL - 1)
                post_q.append(make_mask_exp(e_tile, ex_row, ssum4, atp_box,
                                            bi, inline_atp))
                post_q.append(make_softmax(ssum4, rinv))
                post_q.append(make_tail(bi, ex_row, rinv, atp_box, nat_kc,
                                        inline_atp))
            while post_q:
                post_q.pop(0)()

    nc.compile()
    return nc


def _get_runner():
    if "runner" in _CACHE:
        return _CACHE["runner"]

    import jax
    from jax.sharding import Mesh, PartitionSpec
    from jax.experimental.shard_map import shard_map
    from concourse import bass2jax
    from concourse import mybir as _mb

    nc = _build_nc()
    bass2jax.install_neuronx_cc_hook()

    partition_name = (nc.partition_id_tensor.name
                      if nc.partition_id_tensor else None)
    in_names, out_names, out_avals, zero_outs = [], [], [], []
    for alloc in nc.m.functions[0].allocations:
        if not isinstance(alloc, _mb.MemoryLocationSet):
            continue
        name = alloc.memorylocations[0].name
        if alloc.kind == "ExternalInput":
            if name != partition_name:
                in_names.append(name)
        elif alloc.kind == "ExternalOutput":
            out_names.append(name)
            shape = tuple(alloc.tensor_shape)
            npdt = _mb.dt.np(alloc.dtype)
            out_avals.append(jax.core.ShapedArray(shape, npdt))
            zero_outs.append(np.zeros(shape, npdt))
    n_params = len(in_names)
    n_outs = len(out_names)
    all_in_names = in_names + out_names
    if partition_name is not None:
        all_in_names = all_in_names + [partition_name]
    donate = tuple(range(n_params, n_params + n_outs))

    def _body(*args):
        operands = list(args)
        if partition_name is not None:
            operands.append(bass2jax.partition_id_tensor())
        outs = bass2jax._bass_exec_p.bind(
            *operands,
            out_avals=tuple(out_avals),
            in_names=tuple(all_in_names),
            out_names=tuple(out_names),
            lowering_input_output_aliases=(),
            sim_require_finite=True,
            sim_require_nnan=True,
            nc=nc,
        )
        return tuple(outs)

    devices = jax.devices()[:NCORES]
    mesh = Mesh(np.asarray(devices), ("core",))
    in_specs = (PartitionSpec("core"),) * (n_params + n_outs)
    out_specs = (PartitionSpec("core"),) * n_outs
    sharded = jax.jit(
        shard_map(_body, mesh=mesh, in_specs=in_specs, out_specs=out_specs,
                  check_rep=False),
        donate_argnums=donate, keep_unused=True)

    def run(in_maps):
        concat_in = [
            np.concatenate([np.asarray(m[name]) for m in in_maps], axis=0)
            for name in in_names
        ]
        concat_zeros = [
            np.zeros((NCORES * z.shape[0], *z.shape[1:]), z.dtype)
            for z in zero_outs
        ]
        out_arrs = sharded(*concat_in, *concat_zeros)
        return [
            {name: np.asarray(out_arrs[i]).reshape(NCORES, *out_avals[i].shape)[c]
             for i, name in enumerate(out_names)}
            for c in range(NCORES)
        ]

    _CACHE["runner"] = run
    return run


def _make_in_maps(inputs):
    import ml_dtypes
    bf16 = ml_dtypes.bfloat16

    h_t = np.asarray(inputs["h_t"], dtype=np.float32)
    enc_out = np.asarray(inputs["enc_out"], dtype=np.float32)
    src_mask = np.asarray(inputs["src_mask"])
    Wa = np.asarray(inputs["Wa"], dtype=np.float32)
    Ua = np.asarray(inputs["Ua"], dtype=np.float32)
    va = np.asarray(inputs["va"], dtype=np.float32)

    # [KT, P, HT, P] column blocks of Ua.T / Wa.T (lhsT layouts)
    uaT = np.ascontiguousarray(
        Ua.T.reshape(HT, P, KT, P).transpose(2, 1, 0, 3)).astype(bf16)
    waT = np.ascontiguousarray(
        Wa.T.reshape(HT, P, KT, P).transpose(2, 1, 0, 3)).astype(bf16)
    va_pk = np.ascontiguousarray(va.reshape(KT, P).T).astype(bf16)   # [P, KT]
    encT = np.ascontiguousarray(
        enc_out.transpose(0, 2, 1).reshape(B, HT, P, TC, NT)
        .transpose(0, 3, 2, 1, 4)).astype(bf16)                 # [B, TC, P, HT, NT]
    encn = np.ascontiguousarray(
        enc_out.reshape(B, TT, P, TC, NT)
        .transpose(0, 3, 2, 1, 4)).astype(bf16)                 # [B, TC, P, TT, NT]
    mask_u8 = np.ascontiguousarray(src_mask.astype(np.uint8))

    in_maps = []
    for c in range(NCORES):
        sl = slice(c * BL, (c + 1) * BL)
        htT = np.ascontiguousarray(
            h_t[sl].T.reshape(HT, P, BL).transpose(1, 0, 2)
            .reshape(P, HT * BL)).astype(bf16)                  # [P, HT*BL]
        wall = np.empty((P, WX), dtype=bf16)
        wall[:, OFF_UA0:OFF_VA] = uaT[0].reshape(P, HT * P)
        wall[:, OFF_VA:OFF_HTT] = va_pk
        wall[:, OFF_HTT:OFF_WA0] = htT
        wall[:, OFF_WA0:OFF_REST] = waT[0].reshape(P, HT * P)
        for kt in range(1, KT):
            o = OFF_REST + (kt - 1) * 2048
            wall[:, o:o + 1024] = uaT[kt].reshape(P, HT * P)
            wall[:, o + 1024:o + 2048] = waT[kt].reshape(P, HT * P)
        in_maps.append({
            "encT": encT[sl],
            "encn": encn[sl],
            "wall": wall,
            "mask": mask_u8[sl],
        })
    return in_maps


def kernel(h_t, enc_out, src_mask, Wa, Ua, va):
    in_maps = _make_in_maps({
        "h_t": h_t, "enc_out": enc_out, "src_mask": src_mask,
        "Wa": Wa, "Ua": Ua, "va": va,
    })
    run = _get_runner()
    results = run(in_maps)
    context = np.concatenate([r["ctx"] for r in results], axis=0)
    attn = np.concatenate([r["attn"] for r in results], axis=0)
    return context, attn
